# revision 1
# baseline (speedup 1.0000x reference)
"""GQA attention (B=1, T=2048, D=2048, 32 q heads / 8 kv heads, DH=64, RoPE,
causal) on 8 Trainium2 NeuronCores, tensor-parallel over heads.

Per core: 1 kv head + its 4 q heads (2 pairs). Kernel computes, per core,
partial = (softmax(rope(Q) rope(K)^T / 8) V) @ Wo_shard ; host sums partials.

Layout strategy (all on-chip matmuls contract over the partition dim):
  - host supplies x^T [D, T] so QKV projections use Wq/Wkv as lhsT, x^T as rhs
  - scores are built transposed: S^T[ts, tq] = K^T_chunk.T @ Q^T  (no P
    transposes needed for the AV matmul)
  - V' = [V | 1] column gives the softmax denominator for free in row 64 of
    the AV accumulator
  - q-head pairs run concurrently in the PE array via row tile_position
"""

import numpy as np
from contextlib import ExitStack

import concourse.bass as bass
from concourse import bacc
import concourse.mybir as mybir
import concourse.tile as tile
from concourse.bass_utils import run_bass_kernel_spmd
from concourse.masks import make_identity

B, T, D = 1, 2048, 2048
NH, NKV, DH = 32, 8, 64
NCORES = 8
HPC = NH // NCORES      # 4 q heads per core
PAIRS = HPC // 2        # 2
TB = 512                # tq block (one psum bank of fp32)
NTB = T // TB           # 4
NKT = D // 128          # 16 contraction tiles
NTS = T // 128          # 16 ts blocks
SCALE = 1.0 / float(np.sqrt(DH))

F32 = mybir.dt.float32
F32R = mybir.dt.float32r
EXP = mybir.ActivationFunctionType.Exp

_CACHE = {}


def _r(ap):
    return ap.bitcast(F32R)


def build_nc():
    nc = bacc.Bacc(None, target_bir_lowering=False)

    xT = nc.declare_dram_parameter("xT", [D, T], F32R, isOutput=False)
    wq = nc.declare_dram_parameter("wq", [128, PAIRS, NKT, 128], F32R, isOutput=False)
    wkv = nc.declare_dram_parameter("wkv", [128, NKT, 128], F32R, isOutput=False)
    wo = nc.declare_dram_parameter("wo", [128, 2, T], F32R, isOutput=False)
    tcc = nc.declare_dram_parameter("tcc", [128, T], F32, isOutput=False)
    tss = nc.declare_dram_parameter("tss", [128, T], F32, isOutput=False)
    bm = nc.declare_dram_parameter("bm", [128, 896], F32, isOutput=False)
    out = nc.declare_dram_parameter("out", [T, D], F32, isOutput=True)

    with tile.TileContext(nc) as tc, ExitStack() as top:
        per = top.enter_context(tc.tile_pool(name="persist", bufs=1))

        wq_sb = per.tile([128, PAIRS, NKT, 128], F32R, tag="wq")
        wkv_sb = per.tile([128, NKT, 128], F32R, tag="wkv")
        wo_sb = per.tile([128, 2, T], F32R, tag="wo")
        tcc_sb = per.tile([128, T], F32, tag="tcc")
        tss_sb = per.tile([128, T], F32, tag="tss")
        bm_sb = per.tile([128, 896], F32, tag="bm")
        ident = per.tile([128, 128], F32, tag="ident")
        ones = per.tile([65, 64], F32, tag="ones")

        qt = [per.tile([128, T], F32, tag=f"qt{g}", name=f"qt{g}") for g in range(PAIRS)]
        kvt = per.tile([128, T], F32, tag="kvt")
        qr = [per.tile([128, T], F32R, tag=f"qr{g}", name=f"qr{g}") for g in range(PAIRS)]
        kk = per.tile([128, T], F32R, tag="kk")
        v_sb = per.tile([128, NTS, 65], F32R, tag="v")
        yt = [per.tile([128, T], F32R, tag=f"yt{g}", name=f"yt{g}") for g in range(PAIRS)]

        nc.sync.dma_start(out=wq_sb, in_=wq[:])
        nc.sync.dma_start(out=wkv_sb, in_=wkv[:])
        nc.sync.dma_start(out=wo_sb, in_=wo[:])
        nc.sync.dma_start(out=tcc_sb, in_=tcc[:])
        nc.sync.dma_start(out=tss_sb, in_=tss[:])
        nc.sync.dma_start(out=bm_sb, in_=bm[:])
        make_identity(nc, ident)
        nc.vector.memset(ones, 1.0)

        # ---- Stage A: QKV projections -> Q^T pairs [128,T], [K^T;V^T] [128,T]
        with (
            tc.tile_pool(name="psA", bufs=4, space="PSUM") as psA,
            tc.tile_pool(name="xs", bufs=4) as xs,
            tc.tile_pool(name="psV", bufs=2, space="PSUM") as psV,
            tc.tile_pool(name="rtmp", bufs=2) as rt,
        ):
            for tb in range(NTB):
                accs = [psA.tile([128, TB], F32, tag="acc", name=f"acc{tb}_{i}") for i in range(3)]
                for kt in range(NKT):
                    xt = xs.tile([128, TB], F32R, tag="x")
                    nc.sync.dma_start(
                        out=xt,
                        in_=xT[kt * 128:(kt + 1) * 128, tb * TB:(tb + 1) * TB],
                    )
                    st, sp = kt == 0, kt == NKT - 1
                    nc.tensor.matmul(accs[0], (wq_sb[:, 0, kt]), (xt), start=st, stop=sp)
                    nc.tensor.matmul(accs[1], (wq_sb[:, 1, kt]), (xt), start=st, stop=sp)
                    nc.tensor.matmul(accs[2], (wkv_sb[:, kt]), (xt), start=st, stop=sp)
                cs = slice(tb * TB, (tb + 1) * TB)
                nc.vector.tensor_copy(qt[0][:, cs], accs[0])
                nc.vector.tensor_copy(qt[1][:, cs], accs[1])
                nc.scalar.copy(kvt[:, cs], accs[2])

            # ---- Stage B: RoPE on Q pairs and K; duplicate K; transpose V
            # rotate-half via partition-swap DMA, then qr = q*cc + rot(q)*ss
            # with the signs folded into the ss table (all ops same-partition).
            for g in range(PAIRS):
                rot = rt.tile([128, T], F32, tag="rot")
                for b in range(0, 128, 64):
                    nc.sync.dma_start(out=rot[b:b + 32], in_=qt[g][b + 32:b + 64])
                    nc.sync.dma_start(out=rot[b + 32:b + 64], in_=qt[g][b:b + 32])
                p1 = rt.tile([128, T], F32, tag="p1")
                nc.vector.tensor_mul(p1, qt[g], tcc_sb)
                nc.vector.tensor_mul(rot, rot, tss_sb)
                nc.vector.tensor_add(qr[g], p1, rot)
            rot = rt.tile([128, T], F32, tag="rot")
            nc.sync.dma_start(out=rot[0:32], in_=kvt[32:64])
            nc.sync.dma_start(out=rot[32:64], in_=kvt[0:32])
            p1 = rt.tile([128, T], F32, tag="p1")
            nc.vector.tensor_mul(p1[0:64], kvt[0:64], tcc_sb[0:64])
            nc.vector.tensor_mul(rot[0:64], rot[0:64], tss_sb[0:64])
            nc.vector.tensor_add(kk[0:64], p1[0:64], rot[0:64])
            nc.sync.dma_start(out=kk[64:128], in_=kk[0:64])

            for tt in range(NTS):
                vp = psV.tile([128, 64], F32, tag="vt")
                nc.tensor.transpose(
                    vp, kvt[64:128, tt * 128:(tt + 1) * 128], ident[64:128, 64:128]
                )
                nc.vector.tensor_copy(v_sb[:, tt, 0:64], vp)
            onesv = per.tile([128, NTS, 1], F32, tag="onesv")
            nc.vector.memset(onesv, 1.0)
            nc.vector.tensor_copy(v_sb[:, :, 64:65], onesv)

        # ---- Stage C: attention (ACT-bound) with stage-D matmuls
        # interleaved into the PE bubbles left by exp waits.
        with (
            tc.tile_pool(name="psS", bufs=2, space="PSUM") as psS,
            tc.tile_pool(name="psY", bufs=2, space="PSUM") as psY,
            tc.tile_pool(name="psO", bufs=1, space="PSUM") as psO,
            tc.tile_pool(name="psB", bufs=1, space="PSUM") as psB,
            tc.tile_pool(name="esb", bufs=3) as esb,
            tc.tile_pool(name="rsb", bufs=2) as rsb,
            tc.tile_pool(name="osb", bufs=3) as osb,
        ):
            ready = []

            def emit_d():
                tt, nb = ready.pop(0)
                tsl = slice(tt * 128, (tt + 1) * 128)
                nsl = slice(nb * TB, (nb + 1) * TB)
                po = psO.tile([128, TB], F32, tag="o", name=f"po{tt}_{nb}")
                nc.tensor.matmul(po, yt[0][:, tsl], wo_sb[:, 0, nsl],
                                 start=True, stop=False)
                nc.tensor.matmul(po, yt[1][:, tsl], wo_sb[:, 1, nsl],
                                 start=False, stop=True)
                ob = osb.tile([128, TB], F32, tag="ob", name=f"ob{tt}_{nb}")
                nc.vector.tensor_copy(ob, po)
                nc.sync.dma_start(out=out[tsl, nsl], in_=ob)

            it = 0
            for tb in range(NTB):
                for g in range(PAIRS):
                    ya = psY.tile([65, TB], F32, tag="y", name=f"ya{g}_{tb}")
                    yb = psY.tile([65, TB], F32, tag="y", name=f"yb{g}_{tb}")
                    qs = slice(tb * TB, (tb + 1) * TB)
                    nts_here = 4 * tb + 4
                    for ts in range(nts_here):
                        ks = slice(ts * 128, (ts + 1) * 128)
                        sp = psS.tile([128, 2 * TB], F32, tag="s",
                                      name=f"s{g}_{tb}_{ts}")
                        nc.tensor.matmul(
                            sp[:, 0:TB], kk[0:64, ks], qr[g][0:64, qs],
                            start=True, stop=True, tile_position=(0, 0),
                        )
                        nc.tensor.matmul(
                            sp[:, TB:2 * TB], kk[64:128, ks], qr[g][64:128, qs],
                            start=True, stop=True, tile_position=(64, 0),
                        )
                        e = esb.tile([128, 2 * TB], F32R, tag="e",
                                     name=f"e{g}_{tb}_{ts}")
                        nc.scalar.activation(e, sp, EXP, scale=SCALE)
                        o = ts - 4 * tb
                        if o >= 0:
                            ms = slice((3 - o) * 128, (3 - o) * 128 + TB)
                            nc.vector.tensor_mul(e[:, 0:TB], e[:, 0:TB], bm_sb[:, ms])
                            nc.vector.tensor_mul(e[:, TB:2 * TB], e[:, TB:2 * TB],
                                                 bm_sb[:, ms])
                        st, last = ts == 0, ts == nts_here - 1
                        nc.tensor.matmul(ya, v_sb[:, ts], e[:, 0:TB],
                                         start=st, stop=last)
                        nc.tensor.matmul(yb, v_sb[:, ts], e[:, TB:2 * TB],
                                         start=st, stop=last)
                        if ready and it % 2 == 0:
                            emit_d()
                        it += 1
                    for h, yp in ((0, ya), (1, yb)):
                        rec = rsb.tile([65, TB], F32, tag="rec",
                                       name=f"rec{g}_{tb}_{h}")
                        nc.vector.reciprocal(rec[64:65], yp[64:65, :])
                        bc = psB.tile([64, TB], F32, tag="bc",
                                      name=f"bc{g}_{tb}_{h}")
                        # fp32 (exact) broadcast of 1/l across partitions
                        nc.tensor.matmul(bc, ones[64:65], rec[64:65],
                                         start=True, stop=True)
                        bcs = rsb.tile([64, TB], F32, tag="bcs",
                                       name=f"bcs{g}_{tb}_{h}")
                        nc.vector.tensor_copy(bcs, bc)
                        if h == 0:
                            nc.vector.tensor_mul(yt[g][0:64, qs], yp[0:64, :], bcs)
                        else:
                            yn = rsb.tile([64, TB], F32R, tag="yn",
                                          name=f"yn{g}_{tb}")
                            nc.vector.tensor_mul(yn, yp[0:64, :], bcs)
                            nc.sync.dma_start(out=yt[g][64:128, qs], in_=yn)
                for tt in range(4 * tb, 4 * tb + 4):
                    for nb in range(NTB):
                        ready.append((tt, nb))
            while ready:
                emit_d()

    nc.compile()
    if not nc.is_finalized():
        nc.finalize()
    return nc


def _prep_inputs(x, rc, rs, Wq, Wk, Wv, Wo):
    xT = np.ascontiguousarray(x.reshape(T, D).T).astype(np.float32)
    csT = np.ascontiguousarray(rc.T).astype(np.float32)   # [32, T]
    snT = np.ascontiguousarray(rs.T).astype(np.float32)
    tcc = np.ascontiguousarray(np.concatenate([csT, csT, csT, csT], 0))
    tss = np.ascontiguousarray(np.concatenate([-snT, snT, -snT, snT], 0))
    u = np.arange(896)[None, :]
    p = np.arange(128)[:, None]
    bm = (u >= p + 384).astype(np.float32)

    in_maps = []
    for c in range(NCORES):
        wq_c = Wq[:, c * 256:(c + 1) * 256]               # [D, 256]
        wq_t = np.ascontiguousarray(
            wq_c.reshape(NKT, 128, PAIRS, 128).transpose(1, 2, 0, 3)
        ).astype(np.float32)
        wkv_c = np.concatenate(
            [Wk[:, c * 64:(c + 1) * 64], Wv[:, c * 64:(c + 1) * 64]], 1
        )                                                  # [D, 128]
        wkv_t = np.ascontiguousarray(
            wkv_c.reshape(NKT, 128, 128).transpose(1, 0, 2)
        ).astype(np.float32)
        wo_c = Wo[c * 256:(c + 1) * 256, :]                # [256, D]
        wo_t = np.ascontiguousarray(
            wo_c.reshape(2, 128, T).transpose(1, 0, 2)
        ).astype(np.float32)
        in_maps.append(
            dict(xT=xT, wq=wq_t, wkv=wkv_t, wo=wo_t, tcc=tcc, tss=tss, bm=bm)
        )
    return in_maps


def kernel(x, rc, rs, Wq, Wk, Wv, Wo, _trace=False, _trace_kwargs=None):
    x = np.asarray(x, np.float32)
    if "nc" not in _CACHE:
        _CACHE["nc"] = build_nc()
    nc = _CACHE["nc"]
    in_maps = _prep_inputs(x, rc, rs, np.asarray(Wq), np.asarray(Wk),
                           np.asarray(Wv), np.asarray(Wo))
    kw = {}
    if _trace:
        kw = dict(trace=True, **(_trace_kwargs or {}))
    res = run_bass_kernel_spmd(nc, in_maps, list(range(NCORES)), **kw)
    parts = np.stack([res.results[i]["out"] for i in range(NCORES)])
    full = parts.sum(0, dtype=np.float64).astype(np.float32)
    kernel.last_result = res
    return full.reshape(B, T, D)



# revision 18
# speedup vs baseline: 1.5761x; 1.5761x over previous
"""GQA attention (B=1, T=2048, D=2048, 32 q heads / 8 kv heads, DH=64, RoPE,
causal) on 8 Trainium2 NeuronCores, tensor-parallel over heads.

Per core: 1 kv head + its 4 q heads (2 pairs). Kernel computes, per core,
partial = (softmax(rope(Q) rope(K)^T / 8) V) @ Wo_shard ; host sums partials.

v2 layout/scheduling notes:
  - all matmul operands in bf16 (fp32 PSUM accumulate): same PE rate as
    fp32r but half the DMA/SBUF traffic and 2x DVE on elementwise ops
  - scores are built transposed: S^T[ts, tq] = K^T_chunk.T @ Q^T so the
    AV matmul needs no transposes; V' = [V | 1] gives the softmax
    denominator in row 64 of the AV accumulator
  - stage C is software-pipelined: S(ts+1) issues before AV(ts) so the
    exp latency hides behind PE work; Wo matmuls interleave as fillers
    and stream straight from PSUM to DRAM via DMA
  - normalize: reciprocal_approx_fast (DVE) + partition_broadcast
    (gpsimd) keeps the PE out of the broadcast business
"""

import numpy as np
from contextlib import ExitStack

import concourse.bass as bass
from concourse import bacc
import concourse.mybir as mybir
import concourse.tile as tile
from concourse.bass_utils import run_bass_kernel_spmd
from concourse.masks import make_identity

B, T, D = 1, 2048, 2048
NH, NKV, DH = 32, 8, 64
NCORES = 8
HPC = NH // NCORES      # 4 q heads per core
PAIRS = HPC // 2        # 2
TB = 512                # tq block (one psum bank of fp32)
NTB = T // TB           # 4
NKT = D // 128          # 16 contraction tiles
NTS = T // 128          # 16 ts blocks
SCALE = 1.0 / float(np.sqrt(DH))

F32 = mybir.dt.float32
BF16 = mybir.dt.bfloat16
EXP = mybir.ActivationFunctionType.Exp

_CACHE = {}


def build_nc():
    nc = bacc.Bacc(None, target_bir_lowering=False)

    xT = nc.declare_dram_parameter("xT", [D, T], BF16, isOutput=False)
    wq = nc.declare_dram_parameter("wq", [128, PAIRS, NKT, 128], BF16, isOutput=False)
    wkv = nc.declare_dram_parameter("wkv", [128, NKT, 128], BF16, isOutput=False)
    wo = nc.declare_dram_parameter("wo", [128, 2, T], BF16, isOutput=False)
    tcc = nc.declare_dram_parameter("tcc", [128, T], BF16, isOutput=False)
    tss = nc.declare_dram_parameter("tss", [128, T], BF16, isOutput=False)
    bm = nc.declare_dram_parameter("bm", [128, 896], BF16, isOutput=False)
    out = nc.declare_dram_parameter("out", [T, D], BF16, isOutput=True)

    with tile.TileContext(nc) as tc, ExitStack() as top:
        per = top.enter_context(tc.tile_pool(name="persist", bufs=1))

        wq_sb = per.tile([128, PAIRS, NKT, 128], BF16, tag="wq")
        wkv_sb = per.tile([128, NKT, 128], BF16, tag="wkv")
        wo_sb = per.tile([128, 2, T], BF16, tag="wo")
        tcc_sb = per.tile([128, T], BF16, tag="tcc")
        tss_sb = per.tile([128, T], BF16, tag="tss")
        bm_sb = per.tile([128, 896], BF16, tag="bm")

        qt = [per.tile([128, T], BF16, tag=f"qt{g}", name=f"qt{g}") for g in range(PAIRS)]
        kvt = per.tile([128, T], BF16, tag="kvt")
        qr = [per.tile([128, T], BF16, tag=f"qr{g}", name=f"qr{g}") for g in range(PAIRS)]
        kk = per.tile([128, T], BF16, tag="kk")
        v_sb = per.tile([128, NTS, 65], BF16, tag="v")
        yt = [per.tile([128, T], BF16, tag=f"yt{g}", name=f"yt{g}") for g in range(PAIRS)]

        nc.sync.dma_start(out=wq_sb, in_=wq[:])
        nc.sync.dma_start(out=wkv_sb, in_=wkv[:])
        nc.sync.dma_start(out=wo_sb, in_=wo[:])
        nc.sync.dma_start(out=tcc_sb, in_=tcc[:])
        nc.sync.dma_start(out=tss_sb, in_=tss[:])
        nc.sync.dma_start(out=bm_sb, in_=bm[:])
        onesv = per.tile([128, NTS, 1], BF16, tag="onesv")
        nc.vector.memset(onesv, 1.0)
        nc.vector.tensor_copy(v_sb[:, :, 64:65], onesv)
        ones = per.tile([65, 64], BF16, tag="ones")
        nc.vector.memset(ones, 1.0)
        ident = per.tile([128, 128], BF16, tag="ident")
        make_identity(nc, ident)

        # ---- Stage A+B: QKV projections + RoPE + V transpose, per tq block
        with (
            tc.tile_pool(name="psA", bufs=6, space="PSUM") as psA,
            tc.tile_pool(name="psV", bufs=2, space="PSUM") as psV,
            tc.tile_pool(name="xs", bufs=6) as xs,
            tc.tile_pool(name="rtmp", bufs=2) as rt,
        ):
            for tb in range(NTB):
                accs = [psA.tile([128, TB], F32, tag="acc", name=f"acc{tb}_{i}")
                        for i in range(3)]
                for kt in range(NKT):
                    xt = xs.tile([128, TB], BF16, tag="x")
                    nc.sync.dma_start(
                        out=xt,
                        in_=xT[kt * 128:(kt + 1) * 128, tb * TB:(tb + 1) * TB],
                    )
                    st, sp = kt == 0, kt == NKT - 1
                    nc.tensor.matmul(accs[0], (wq_sb[:, 0, kt]), (xt), start=st, stop=sp)
                    nc.tensor.matmul(accs[1], (wq_sb[:, 1, kt]), (xt), start=st, stop=sp)
                    nc.tensor.matmul(accs[2], (wkv_sb[:, kt]), (xt), start=st, stop=sp)
                cs = slice(tb * TB, (tb + 1) * TB)
                nc.scalar.copy(qt[0][:, cs], accs[0])
                nc.scalar.copy(qt[1][:, cs], accs[1])
                nc.vector.tensor_copy(kvt[:, cs], accs[2])

                # RoPE on this tq block: rotate-half via partition-swap DMA,
                # qr = q*cc + rot(q)*ss with signs folded into ss.
                for g in range(PAIRS):
                    rot = rt.tile([128, TB], BF16, tag="rot")
                    for b in range(0, 128, 64):
                        nc.sync.dma_start(out=rot[b:b + 32], in_=qt[g][b + 32:b + 64, cs])
                        nc.sync.dma_start(out=rot[b + 32:b + 64], in_=qt[g][b:b + 32, cs])
                    p1 = rt.tile([128, TB], BF16, tag="p1")
                    nc.gpsimd.tensor_mul(p1, qt[g][:, cs], tcc_sb[:, cs])
                    nc.gpsimd.tensor_mul(rot, rot, tss_sb[:, cs])
                    nc.gpsimd.tensor_add(qr[g][:, cs], p1, rot)
                rot = rt.tile([128, TB], BF16, tag="rot")
                nc.sync.dma_start(out=rot[0:32], in_=kvt[32:64, cs])
                nc.sync.dma_start(out=rot[32:64], in_=kvt[0:32, cs])
                p1 = rt.tile([128, TB], BF16, tag="p1")
                nc.gpsimd.tensor_mul(p1[0:64], kvt[0:64, cs], tcc_sb[0:64, cs])
                nc.gpsimd.tensor_mul(rot[0:64], rot[0:64], tss_sb[0:64, cs])
                nc.gpsimd.tensor_add(kk[0:64, cs], p1[0:64], rot[0:64])
                nc.sync.dma_start(out=kk[64:128, cs], in_=kk[0:64, cs])

                # V chunks via PE transpose
                for tt in range(4 * tb, 4 * tb + 4):
                    vp = psV.tile([128, 64], BF16, tag="vt", name=f"vp{tt}")
                    nc.tensor.transpose(
                        vp, kvt[64:128, tt * 128:(tt + 1) * 128],
                        ident[64:128, 64:128],
                    )
                    nc.vector.tensor_copy(v_sb[:, tt, 0:64], vp)

        # ---- Stage C: attention, software-pipelined; Wo matmuls as fillers
        with (
            tc.tile_pool(name="psS", bufs=2, space="PSUM") as psS,
            tc.tile_pool(name="psY", bufs=2, space="PSUM") as psY,
            tc.tile_pool(name="psO", bufs=2, space="PSUM") as psO,
            tc.tile_pool(name="esb", bufs=3) as esb,
            tc.tile_pool(name="rsb", bufs=2) as rsb,
            tc.tile_pool(name="osb", bufs=3) as osb,
        ):
            ready = []

            def emit_d():
                tt, nb = ready.pop(0)
                tsl = slice(tt * 128, (tt + 1) * 128)
                nsl = slice(nb * TB, (nb + 1) * TB)
                po = psO.tile([128, TB], F32, tag="po", name=f"po{tt}_{nb}")
                nc.tensor.matmul(po, yt[0][:, tsl], wo_sb[:, 0, nsl],
                                 start=True, stop=False)
                nc.tensor.matmul(po, yt[1][:, tsl], wo_sb[:, 1, nsl],
                                 start=False, stop=True)
                ob = osb.tile([128, TB], BF16, tag="ob", name=f"ob{tt}_{nb}")
                nc.vector.tensor_copy(ob, po)
                nc.sync.dma_start(out=out[tsl, nsl], in_=ob)

            for tb in range(NTB):
                qs = slice(tb * TB, (tb + 1) * TB)
                nts_here = 4 * tb + 4
                for g in range(PAIRS):
                    ya = psY.tile([65, TB], F32, tag="y", name=f"ya{g}_{tb}")
                    yb = psY.tile([65, TB], F32, tag="y", name=f"yb{g}_{tb}")
                    sps = {}
                    es = {}

                    def emit_s(ts):
                        sp = psS.tile([128, 2 * TB], F32, tag="s",
                                      name=f"s{g}_{tb}_{ts}")
                        nc.tensor.matmul(
                            sp[:, 0:TB], kk[0:64, ts * 128:(ts + 1) * 128],
                            qr[g][0:64, qs],
                            start=True, stop=True, tile_position=(0, 0),
                        )
                        nc.tensor.matmul(
                            sp[:, TB:2 * TB], kk[64:128, ts * 128:(ts + 1) * 128],
                            qr[g][64:128, qs],
                            start=True, stop=True, tile_position=(64, 0),
                        )
                        sps[ts] = sp

                    def emit_e(ts):
                        e = esb.tile([128, 2 * TB], BF16, tag="e",
                                     name=f"e{g}_{tb}_{ts}")
                        nc.scalar.activation(e, sps.pop(ts), EXP, scale=SCALE)
                        o = ts - 4 * tb
                        if o >= 0:
                            ms = slice((3 - o) * 128, (3 - o) * 128 + TB)
                            nc.vector.tensor_mul(e[:, 0:TB], e[:, 0:TB], bm_sb[:, ms])
                            nc.vector.tensor_mul(e[:, TB:2 * TB], e[:, TB:2 * TB],
                                                 bm_sb[:, ms])
                        es[ts] = e

                    def emit_av(ts):
                        e = es.pop(ts)
                        st, last = ts == 0, ts == nts_here - 1
                        nc.tensor.matmul(ya, v_sb[:, ts], e[:, 0:TB],
                                         start=st, stop=last)
                        nc.tensor.matmul(yb, v_sb[:, ts], e[:, TB:2 * TB],
                                         start=st, stop=last)

                    # software pipeline: S runs one step ahead of AV
                    emit_s(0)
                    emit_e(0)
                    for ts in range(1, nts_here):
                        emit_s(ts)
                        emit_e(ts)
                        emit_av(ts - 1)
                        if ready:
                            emit_d()
                    emit_av(nts_here - 1)

                    # normalize: PE-broadcast the raw denominator row, then
                    # one fast reciprocal of the whole broadcast (PSUM->SBUF)
                    # doubles as the copy; finally scale y into yt.
                    for h, yp in ((0, ya), (1, yb)):
                        l16 = rsb.tile([65, TB], BF16, tag="l16",
                                       name=f"l16{g}_{tb}_{h}")
                        nc.scalar.copy(l16[64:65], yp[64:65, :])
                        bc = psO.tile([64, TB], F32, tag="po",
                                      name=f"bc{g}_{tb}_{h}")
                        nc.tensor.matmul(bc, ones[64:65], l16[64:65],
                                         start=True, stop=True)
                        binv = rsb.tile([64, TB], F32, tag="binv",
                                        name=f"binv{g}_{tb}_{h}")
                        nc.vector.reciprocal_approx_fast(binv, bc)
                        if h == 0:
                            nc.vector.tensor_mul(yt[g][0:64, qs], yp[0:64, :], binv)
                        else:
                            yn = rsb.tile([64, TB], BF16, tag="yn",
                                          name=f"yn{g}_{tb}")
                            nc.vector.tensor_mul(yn, yp[0:64, :], binv)
                            nc.sync.dma_start(out=yt[g][64:128, qs], in_=yn)
                for tt in range(4 * tb, 4 * tb + 4):
                    for nb in range(NTB):
                        ready.append((tt, nb))
            while ready:
                emit_d()

    nc.compile()
    if not nc.is_finalized():
        nc.finalize()
    return nc


def _prep_inputs(x, rc, rs, Wq, Wk, Wv, Wo):
    bf16 = mybir.dt.np(BF16)
    xT = np.ascontiguousarray(x.reshape(T, D).T).astype(bf16)
    csT = np.ascontiguousarray(rc.T).astype(np.float32)   # [32, T]
    snT = np.ascontiguousarray(rs.T).astype(np.float32)
    tcc = np.ascontiguousarray(np.concatenate([csT, csT, csT, csT], 0)).astype(bf16)
    tss = np.ascontiguousarray(np.concatenate([-snT, snT, -snT, snT], 0)).astype(bf16)
    u = np.arange(896)[None, :]
    p = np.arange(128)[:, None]
    bm = (u >= p + 384).astype(bf16)

    in_maps = []
    for c in range(NCORES):
        wq_c = Wq[:, c * 256:(c + 1) * 256]               # [D, 256]
        wq_t = np.ascontiguousarray(
            wq_c.reshape(NKT, 128, PAIRS, 128).transpose(1, 2, 0, 3)
        ).astype(bf16)
        wkv_c = np.concatenate(
            [Wk[:, c * 64:(c + 1) * 64], Wv[:, c * 64:(c + 1) * 64]], 1
        )                                                  # [D, 128]
        wkv_t = np.ascontiguousarray(
            wkv_c.reshape(NKT, 128, 128).transpose(1, 0, 2)
        ).astype(bf16)
        wo_c = Wo[c * 256:(c + 1) * 256, :]                # [256, D]
        wo_t = np.ascontiguousarray(
            wo_c.reshape(2, 128, T).transpose(1, 0, 2)
        ).astype(bf16)
        in_maps.append(
            dict(xT=xT, wq=wq_t, wkv=wkv_t, wo=wo_t, tcc=tcc, tss=tss, bm=bm)
        )
    return in_maps


def kernel(x, rc, rs, Wq, Wk, Wv, Wo, _trace=False, _trace_kwargs=None):
    x = np.asarray(x, np.float32)
    if "nc" not in _CACHE:
        _CACHE["nc"] = build_nc()
    nc = _CACHE["nc"]
    in_maps = _prep_inputs(x, rc, rs, np.asarray(Wq), np.asarray(Wk),
                           np.asarray(Wv), np.asarray(Wo))
    kw = {}
    if _trace:
        kw = dict(trace=True, **(_trace_kwargs or {}))
    res = run_bass_kernel_spmd(nc, in_maps, list(range(NCORES)), **kw)
    parts = np.stack(
        [np.asarray(res.results[i]["out"]).astype(np.float32) for i in range(NCORES)]
    )
    full = parts.sum(0, dtype=np.float64).astype(np.float32)
    kernel.last_result = res
    return full.reshape(B, T, D)


# revision 21
# speedup vs baseline: 1.5932x; 1.0108x over previous
"""GQA attention (B=1, T=2048, D=2048, 32 q heads / 8 kv heads, DH=64, RoPE,
causal) on 8 Trainium2 NeuronCores, tensor-parallel over heads.

Per core: 1 kv head + its 4 q heads (2 pairs). Kernel computes, per core,
partial = (softmax(rope(Q) rope(K)^T / 8) V) @ Wo_shard ; host sums partials.

v2 layout/scheduling notes:
  - all matmul operands in bf16 (fp32 PSUM accumulate): same PE rate as
    fp32r but half the DMA/SBUF traffic and 2x DVE on elementwise ops
  - scores are built transposed: S^T[ts, tq] = K^T_chunk.T @ Q^T so the
    AV matmul needs no transposes; V' = [V | 1] gives the softmax
    denominator in row 64 of the AV accumulator
  - stage C is software-pipelined: S(ts+1) issues before AV(ts) so the
    exp latency hides behind PE work; Wo matmuls interleave as fillers
    and stream straight from PSUM to DRAM via DMA
  - normalize: reciprocal_approx_fast (DVE) + partition_broadcast
    (gpsimd) keeps the PE out of the broadcast business
"""

import numpy as np
from contextlib import ExitStack

import concourse.bass as bass
from concourse import bacc
import concourse.mybir as mybir
import concourse.tile as tile
from concourse.bass_utils import run_bass_kernel_spmd
from concourse.masks import make_identity

B, T, D = 1, 2048, 2048
NH, NKV, DH = 32, 8, 64
NCORES = 8
HPC = NH // NCORES      # 4 q heads per core
PAIRS = HPC // 2        # 2
TB = 512                # tq block (one psum bank of fp32)
NTB = T // TB           # 4
NKT = D // 128          # 16 contraction tiles
NTS = T // 128          # 16 ts blocks
SCALE = 1.0 / float(np.sqrt(DH))

F32 = mybir.dt.float32
BF16 = mybir.dt.bfloat16
EXP = mybir.ActivationFunctionType.Exp

_CACHE = {}


def build_nc():
    nc = bacc.Bacc(None, target_bir_lowering=False)

    xT = nc.declare_dram_parameter("xT", [D, T], BF16, isOutput=False)
    wq = nc.declare_dram_parameter("wq", [128, PAIRS, NKT, 128], BF16, isOutput=False)
    wkv = nc.declare_dram_parameter("wkv", [128, NKT, 128], BF16, isOutput=False)
    wo = nc.declare_dram_parameter("wo", [128, 2, T], BF16, isOutput=False)
    tcc = nc.declare_dram_parameter("tcc", [128, T], BF16, isOutput=False)
    tss = nc.declare_dram_parameter("tss", [128, T], BF16, isOutput=False)
    bm = nc.declare_dram_parameter("bm", [128, 896], BF16, isOutput=False)
    out = nc.declare_dram_parameter("out", [T, D], BF16, isOutput=True)

    with tile.TileContext(nc) as tc, ExitStack() as top:
        per = top.enter_context(tc.tile_pool(name="persist", bufs=1))

        wq_sb = per.tile([128, PAIRS, NKT, 128], BF16, tag="wq")
        wkv_sb = per.tile([128, NKT, 128], BF16, tag="wkv")
        wo_sb = per.tile([128, 2, T], BF16, tag="wo")
        tcc_sb = per.tile([128, T], BF16, tag="tcc")
        tss_sb = per.tile([128, T], BF16, tag="tss")
        bm_sb = per.tile([128, 896], BF16, tag="bm")

        qt = [per.tile([128, T], BF16, tag=f"qt{g}", name=f"qt{g}") for g in range(PAIRS)]
        kvt = per.tile([128, T], BF16, tag="kvt")
        qr = [per.tile([128, T], BF16, tag=f"qr{g}", name=f"qr{g}") for g in range(PAIRS)]
        kk = per.tile([128, T], BF16, tag="kk")
        v_sb = per.tile([128, NTS, 65], BF16, tag="v")
        yt = [per.tile([128, T], BF16, tag=f"yt{g}", name=f"yt{g}") for g in range(PAIRS)]

        # wq/wkv gate the first matmuls -- load them first; the rest are
        # staggered into the stage-A loop so x tiles aren't queued behind them
        nc.sync.dma_start(out=wq_sb, in_=wq[:])
        nc.sync.dma_start(out=wkv_sb, in_=wkv[:])
        onesv = per.tile([128, NTS, 1], BF16, tag="onesv")
        nc.vector.memset(onesv, 1.0)
        nc.vector.tensor_copy(v_sb[:, :, 64:65], onesv)
        ones = per.tile([65, 64], BF16, tag="ones")
        nc.vector.memset(ones, 1.0)
        ident = per.tile([128, 128], BF16, tag="ident")
        make_identity(nc, ident)

        # ---- Stage A+B: QKV projections + RoPE + V transpose, per tq block
        with (
            tc.tile_pool(name="psA", bufs=6, space="PSUM") as psA,
            tc.tile_pool(name="psV", bufs=2, space="PSUM") as psV,
            tc.tile_pool(name="xs", bufs=12) as xs,
            tc.tile_pool(name="rtmp", bufs=2) as rt,
        ):
            for tb in range(NTB):
                accs = [psA.tile([128, TB], F32, tag="acc", name=f"acc{tb}_{i}")
                        for i in range(3)]
                for kt in range(NKT):
                    xt = xs.tile([128, TB], BF16, tag="x")
                    nc.sync.dma_start(
                        out=xt,
                        in_=xT[kt * 128:(kt + 1) * 128, tb * TB:(tb + 1) * TB],
                    )
                    st, sp = kt == 0, kt == NKT - 1
                    nc.tensor.matmul(accs[0], (wq_sb[:, 0, kt]), (xt), start=st, stop=sp)
                    nc.tensor.matmul(accs[1], (wq_sb[:, 1, kt]), (xt), start=st, stop=sp)
                    nc.tensor.matmul(accs[2], (wkv_sb[:, kt]), (xt), start=st, stop=sp)
                if tb == 0:
                    nc.sync.dma_start(out=tcc_sb, in_=tcc[:])
                    nc.sync.dma_start(out=tss_sb, in_=tss[:])
                elif tb == 1:
                    nc.sync.dma_start(out=bm_sb, in_=bm[:])
                    nc.sync.dma_start(out=wo_sb, in_=wo[:])
                cs = slice(tb * TB, (tb + 1) * TB)
                nc.scalar.copy(qt[0][:, cs], accs[0])
                nc.scalar.copy(qt[1][:, cs], accs[1])
                nc.vector.tensor_copy(kvt[:, cs], accs[2])

                # RoPE on this tq block: rotate-half via partition-swap DMA,
                # qr = q*cc + rot(q)*ss with signs folded into ss.
                for g in range(PAIRS):
                    rot = rt.tile([128, TB], BF16, tag="rot")
                    for b in range(0, 128, 64):
                        nc.sync.dma_start(out=rot[b:b + 32], in_=qt[g][b + 32:b + 64, cs])
                        nc.sync.dma_start(out=rot[b + 32:b + 64], in_=qt[g][b:b + 32, cs])
                    p1 = rt.tile([128, TB], BF16, tag="p1")
                    nc.gpsimd.tensor_mul(p1, qt[g][:, cs], tcc_sb[:, cs])
                    nc.gpsimd.tensor_mul(rot, rot, tss_sb[:, cs])
                    nc.gpsimd.tensor_add(qr[g][:, cs], p1, rot)
                rot = rt.tile([128, TB], BF16, tag="rot")
                nc.sync.dma_start(out=rot[0:32], in_=kvt[32:64, cs])
                nc.sync.dma_start(out=rot[32:64], in_=kvt[0:32, cs])
                p1 = rt.tile([128, TB], BF16, tag="p1")
                nc.gpsimd.tensor_mul(p1[0:64], kvt[0:64, cs], tcc_sb[0:64, cs])
                nc.gpsimd.tensor_mul(rot[0:64], rot[0:64], tss_sb[0:64, cs])
                nc.gpsimd.tensor_add(kk[0:64, cs], p1[0:64], rot[0:64])
                nc.sync.dma_start(out=kk[64:128, cs], in_=kk[0:64, cs])

                # V chunks via PE transpose
                for tt in range(4 * tb, 4 * tb + 4):
                    vp = psV.tile([128, 64], BF16, tag="vt", name=f"vp{tt}")
                    nc.tensor.transpose(
                        vp, kvt[64:128, tt * 128:(tt + 1) * 128],
                        ident[64:128, 64:128],
                    )
                    nc.vector.tensor_copy(v_sb[:, tt, 0:64], vp)

        # ---- Stage C: attention, software-pipelined; Wo matmuls as fillers
        with (
            tc.tile_pool(name="psS", bufs=2, space="PSUM") as psS,
            tc.tile_pool(name="psY", bufs=2, space="PSUM") as psY,
            tc.tile_pool(name="psO", bufs=2, space="PSUM") as psO,
            tc.tile_pool(name="esb", bufs=3) as esb,
            tc.tile_pool(name="rsb", bufs=2) as rsb,
            tc.tile_pool(name="osb", bufs=3) as osb,
        ):
            ready = []

            def emit_d():
                tt, nb = ready.pop(0)
                tsl = slice(tt * 128, (tt + 1) * 128)
                nsl = slice(nb * TB, (nb + 1) * TB)
                po = psO.tile([128, TB], F32, tag="po", name=f"po{tt}_{nb}")
                nc.tensor.matmul(po, yt[0][:, tsl], wo_sb[:, 0, nsl],
                                 start=True, stop=False)
                nc.tensor.matmul(po, yt[1][:, tsl], wo_sb[:, 1, nsl],
                                 start=False, stop=True)
                ob = osb.tile([128, TB], BF16, tag="ob", name=f"ob{tt}_{nb}")
                nc.vector.tensor_copy(ob, po)
                nc.sync.dma_start(out=out[tsl, nsl], in_=ob)

            for tb in range(NTB):
                qs = slice(tb * TB, (tb + 1) * TB)
                nts_here = 4 * tb + 4
                for g in range(PAIRS):
                    ya = psY.tile([65, TB], F32, tag="y", name=f"ya{g}_{tb}")
                    yb = psY.tile([65, TB], F32, tag="y", name=f"yb{g}_{tb}")
                    sps = {}
                    es = {}

                    def emit_s(ts):
                        # diagonal blocks (o>0) only need tq cols >= 128*o
                        o = max(0, ts - 4 * tb)
                        w = 128 * o
                        sub = slice(tb * TB + w, (tb + 1) * TB)
                        sp = psS.tile([128, 2 * TB], F32, tag="s",
                                      name=f"s{g}_{tb}_{ts}")
                        nc.tensor.matmul(
                            sp[:, w:TB], kk[0:64, ts * 128:(ts + 1) * 128],
                            qr[g][0:64, sub],
                            start=True, stop=True, tile_position=(0, 0),
                        )
                        nc.tensor.matmul(
                            sp[:, TB + w:2 * TB], kk[64:128, ts * 128:(ts + 1) * 128],
                            qr[g][64:128, sub],
                            start=True, stop=True, tile_position=(64, 0),
                        )
                        sps[ts] = sp

                    def emit_e(ts):
                        o = ts - 4 * tb
                        w = 128 * max(0, o)
                        sp = sps.pop(ts)
                        e = esb.tile([128, 2 * TB], BF16, tag="e",
                                     name=f"e{g}_{tb}_{ts}")
                        if w == 0:
                            nc.scalar.activation(e, sp, EXP, scale=SCALE)
                        else:
                            nc.scalar.activation(e[:, w:TB], sp[:, w:TB],
                                                 EXP, scale=SCALE)
                            nc.scalar.activation(e[:, TB + w:2 * TB],
                                                 sp[:, TB + w:2 * TB],
                                                 EXP, scale=SCALE)
                            # masked-out strips must read as zero in the AV
                            nc.gpsimd.memset(e[:, 0:w], 0.0)
                            nc.gpsimd.memset(e[:, TB:TB + w], 0.0)
                        if o >= 0:
                            # the partial 128-wide diagonal sub-block
                            nc.vector.tensor_mul(e[:, w:w + 128], e[:, w:w + 128],
                                                 bm_sb[:, 384:512])
                            nc.vector.tensor_mul(e[:, TB + w:TB + w + 128],
                                                 e[:, TB + w:TB + w + 128],
                                                 bm_sb[:, 384:512])
                        es[ts] = e

                    def emit_av(ts):
                        e = es.pop(ts)
                        st, last = ts == 0, ts == nts_here - 1
                        nc.tensor.matmul(ya, v_sb[:, ts], e[:, 0:TB],
                                         start=st, stop=last)
                        nc.tensor.matmul(yb, v_sb[:, ts], e[:, TB:2 * TB],
                                         start=st, stop=last)

                    # software pipeline: S one step ahead of AV; Wo fillers
                    # issue before the exp-waiting AV so the PE never stalls
                    emit_s(0)
                    emit_e(0)
                    for ts in range(1, nts_here):
                        emit_s(ts)
                        emit_e(ts)
                        if ready and (tb < 3 or len(ready) > 4):
                            emit_d()
                        emit_av(ts - 1)
                    emit_av(nts_here - 1)

                    # normalize: PE-broadcast the raw denominator row, then
                    # one fast reciprocal of the whole broadcast (PSUM->SBUF)
                    # doubles as the copy; finally scale y into yt.
                    for h, yp in ((0, ya), (1, yb)):
                        l16 = rsb.tile([65, TB], BF16, tag="l16",
                                       name=f"l16{g}_{tb}_{h}")
                        nc.scalar.copy(l16[64:65], yp[64:65, :])
                        bc = psO.tile([64, TB], F32, tag="po",
                                      name=f"bc{g}_{tb}_{h}")
                        nc.tensor.matmul(bc, ones[64:65], l16[64:65],
                                         start=True, stop=True)
                        binv = rsb.tile([64, TB], F32, tag="binv",
                                        name=f"binv{g}_{tb}_{h}")
                        nc.vector.reciprocal_approx_fast(binv, bc)
                        if h == 0:
                            nc.vector.tensor_mul(yt[g][0:64, qs], yp[0:64, :], binv)
                        else:
                            yn = rsb.tile([64, TB], BF16, tag="yn",
                                          name=f"yn{g}_{tb}")
                            nc.vector.tensor_mul(yn, yp[0:64, :], binv)
                            nc.sync.dma_start(out=yt[g][64:128, qs], in_=yn)
                for tt in range(4 * tb, 4 * tb + 4):
                    for nb in range(NTB):
                        ready.append((tt, nb))
            while ready:
                emit_d()

    nc.compile()
    if not nc.is_finalized():
        nc.finalize()
    return nc


def _prep_inputs(x, rc, rs, Wq, Wk, Wv, Wo):
    bf16 = mybir.dt.np(BF16)
    xT = np.ascontiguousarray(x.reshape(T, D).T).astype(bf16)
    csT = np.ascontiguousarray(rc.T).astype(np.float32)   # [32, T]
    snT = np.ascontiguousarray(rs.T).astype(np.float32)
    tcc = np.ascontiguousarray(np.concatenate([csT, csT, csT, csT], 0)).astype(bf16)
    tss = np.ascontiguousarray(np.concatenate([-snT, snT, -snT, snT], 0)).astype(bf16)
    u = np.arange(896)[None, :]
    p = np.arange(128)[:, None]
    bm = (u >= p + 384).astype(bf16)

    in_maps = []
    for c in range(NCORES):
        wq_c = Wq[:, c * 256:(c + 1) * 256]               # [D, 256]
        wq_t = np.ascontiguousarray(
            wq_c.reshape(NKT, 128, PAIRS, 128).transpose(1, 2, 0, 3)
        ).astype(bf16)
        wkv_c = np.concatenate(
            [Wk[:, c * 64:(c + 1) * 64], Wv[:, c * 64:(c + 1) * 64]], 1
        )                                                  # [D, 128]
        wkv_t = np.ascontiguousarray(
            wkv_c.reshape(NKT, 128, 128).transpose(1, 0, 2)
        ).astype(bf16)
        wo_c = Wo[c * 256:(c + 1) * 256, :]                # [256, D]
        wo_t = np.ascontiguousarray(
            wo_c.reshape(2, 128, T).transpose(1, 0, 2)
        ).astype(bf16)
        in_maps.append(
            dict(xT=xT, wq=wq_t, wkv=wkv_t, wo=wo_t, tcc=tcc, tss=tss, bm=bm)
        )
    return in_maps


def kernel(x, rc, rs, Wq, Wk, Wv, Wo, _trace=False, _trace_kwargs=None):
    x = np.asarray(x, np.float32)
    if "nc" not in _CACHE:
        _CACHE["nc"] = build_nc()
    nc = _CACHE["nc"]
    in_maps = _prep_inputs(x, rc, rs, np.asarray(Wq), np.asarray(Wk),
                           np.asarray(Wv), np.asarray(Wo))
    kw = {}
    if _trace:
        kw = dict(trace=True, **(_trace_kwargs or {}))
    res = run_bass_kernel_spmd(nc, in_maps, list(range(NCORES)), **kw)
    parts = np.stack(
        [np.asarray(res.results[i]["out"]).astype(np.float32) for i in range(NCORES)]
    )
    full = parts.sum(0, dtype=np.float64).astype(np.float32)
    kernel.last_result = res
    return full.reshape(B, T, D)


# revision 26
# speedup vs baseline: 1.6035x; 1.0065x over previous
"""GQA attention (B=1, T=2048, D=2048, 32 q heads / 8 kv heads, DH=64, RoPE,
causal) on 8 Trainium2 NeuronCores, tensor-parallel over heads.

Per core: 1 kv head + its 4 q heads (2 pairs). Kernel computes, per core,
partial = (softmax(rope(Q) rope(K)^T / 8) V) @ Wo_shard ; host sums partials.

v2 layout/scheduling notes:
  - all matmul operands in bf16 (fp32 PSUM accumulate): same PE rate as
    fp32r but half the DMA/SBUF traffic and 2x DVE on elementwise ops
  - scores are built transposed: S^T[ts, tq] = K^T_chunk.T @ Q^T so the
    AV matmul needs no transposes; V' = [V | 1] gives the softmax
    denominator in row 64 of the AV accumulator
  - stage C is software-pipelined: S(ts+1) issues before AV(ts) so the
    exp latency hides behind PE work; Wo matmuls interleave as fillers
    and stream straight from PSUM to DRAM via DMA
  - normalize: reciprocal_approx_fast (DVE) + partition_broadcast
    (gpsimd) keeps the PE out of the broadcast business
"""

import numpy as np
from contextlib import ExitStack

import concourse.bass as bass
from concourse import bacc
import concourse.mybir as mybir
import concourse.tile as tile
from concourse.bass_utils import run_bass_kernel_spmd
from concourse.masks import make_identity

B, T, D = 1, 2048, 2048
NH, NKV, DH = 32, 8, 64
NCORES = 8
HPC = NH // NCORES      # 4 q heads per core
PAIRS = HPC // 2        # 2
TB = 512                # tq block (one psum bank of fp32)
NTB = T // TB           # 4
NKT = D // 128          # 16 contraction tiles
NTS = T // 128          # 16 ts blocks
SCALE = 1.0 / float(np.sqrt(DH))

F32 = mybir.dt.float32
BF16 = mybir.dt.bfloat16
EXP = mybir.ActivationFunctionType.Exp

_CACHE = {}


def build_nc():
    nc = bacc.Bacc(None, target_bir_lowering=False)

    xT = nc.declare_dram_parameter("xT", [D, T], BF16, isOutput=False)
    wq = nc.declare_dram_parameter("wq", [128, PAIRS, NKT, 128], BF16, isOutput=False)
    wkv = nc.declare_dram_parameter("wkv", [128, NKT, 128], BF16, isOutput=False)
    wo = nc.declare_dram_parameter("wo", [128, 2, T], BF16, isOutput=False)
    tcc = nc.declare_dram_parameter("tcc", [128, T], BF16, isOutput=False)
    tss = nc.declare_dram_parameter("tss", [128, T], BF16, isOutput=False)
    bm = nc.declare_dram_parameter("bm", [128, 896], BF16, isOutput=False)
    out = nc.declare_dram_parameter("out", [T, D], BF16, isOutput=True)

    with tile.TileContext(nc) as tc, ExitStack() as top:
        per = top.enter_context(tc.tile_pool(name="persist", bufs=1))

        wq_sb = per.tile([128, PAIRS, NKT, 128], BF16, tag="wq")
        wkv_sb = per.tile([128, NKT, 128], BF16, tag="wkv")
        wo_sb = per.tile([128, 2, T], BF16, tag="wo")
        tcc_sb = per.tile([128, T], BF16, tag="tcc")
        tss_sb = per.tile([128, T], BF16, tag="tss")
        bm_sb = per.tile([128, 896], BF16, tag="bm")

        qt = [per.tile([128, T], BF16, tag=f"qt{g}", name=f"qt{g}") for g in range(PAIRS)]
        kvt = per.tile([128, T], BF16, tag="kvt")
        qr = [per.tile([128, T], BF16, tag=f"qr{g}", name=f"qr{g}") for g in range(PAIRS)]
        kk = per.tile([128, T], BF16, tag="kk")
        v_sb = per.tile([128, NTS, 65], BF16, tag="v")
        yt = [per.tile([128, T], BF16, tag=f"yt{g}", name=f"yt{g}") for g in range(PAIRS)]

        # wq/wkv gate the first matmuls -- load them first, per contraction
        # tile so matmul kt=0 can start before the whole weight arrives; the
        # rest are staggered into the stage-A loop so x tiles aren't queued
        # behind them
        for kt in range(NKT):
            nc.sync.dma_start(out=wq_sb[:, :, kt], in_=wq[:, :, kt])
            nc.sync.dma_start(out=wkv_sb[:, kt], in_=wkv[:, kt])
        onesv = per.tile([128, NTS, 1], BF16, tag="onesv")
        nc.vector.memset(onesv, 1.0)
        nc.vector.tensor_copy(v_sb[:, :, 64:65], onesv)
        ones = per.tile([65, 64], BF16, tag="ones")
        nc.vector.memset(ones, 1.0)
        ident = per.tile([128, 128], BF16, tag="ident")
        make_identity(nc, ident)

        # ---- Stage A+B: QKV projections + RoPE + V transpose, per tq block
        with (
            tc.tile_pool(name="psA", bufs=6, space="PSUM") as psA,
            tc.tile_pool(name="psV", bufs=2, space="PSUM") as psV,
            tc.tile_pool(name="xs", bufs=12) as xs,
            tc.tile_pool(name="rtmp", bufs=2) as rt,
        ):
            for tb in range(NTB):
                accs = [psA.tile([128, TB], F32, tag="acc", name=f"acc{tb}_{i}")
                        for i in range(3)]
                for kt in range(NKT):
                    xt = xs.tile([128, TB], BF16, tag="x")
                    nc.sync.dma_start(
                        out=xt,
                        in_=xT[kt * 128:(kt + 1) * 128, tb * TB:(tb + 1) * TB],
                    )
                    st, sp = kt == 0, kt == NKT - 1
                    nc.tensor.matmul(accs[0], (wq_sb[:, 0, kt]), (xt), start=st, stop=sp)
                    nc.tensor.matmul(accs[1], (wq_sb[:, 1, kt]), (xt), start=st, stop=sp)
                    nc.tensor.matmul(accs[2], (wkv_sb[:, kt]), (xt), start=st, stop=sp)
                if tb == 0:
                    nc.sync.dma_start(out=tcc_sb, in_=tcc[:])
                    nc.sync.dma_start(out=tss_sb, in_=tss[:])
                elif tb == 1:
                    nc.sync.dma_start(out=bm_sb, in_=bm[:])
                    nc.sync.dma_start(out=wo_sb, in_=wo[:])
                cs = slice(tb * TB, (tb + 1) * TB)
                nc.scalar.copy(qt[0][:, cs], accs[0])
                nc.scalar.copy(qt[1][:, cs], accs[1])
                nc.vector.tensor_copy(kvt[:, cs], accs[2])

                # RoPE on this tq block: rotate-half via partition-swap DMA
                # (issued on the ACT DGE so stalled swaps never block the
                # sync-engine x-tile loads), qr = q*cc + rot(q)*ss with the
                # signs folded into ss.
                for g in range(PAIRS):
                    rot = rt.tile([128, TB], BF16, tag="rot", bufs=6)
                    for b in range(0, 128, 64):
                        nc.scalar.dma_start(out=rot[b:b + 32], in_=qt[g][b + 32:b + 64, cs])
                        nc.scalar.dma_start(out=rot[b + 32:b + 64], in_=qt[g][b:b + 32, cs])
                    p1 = rt.tile([128, TB], BF16, tag="p1", bufs=6)
                    nc.gpsimd.tensor_mul(p1, qt[g][:, cs], tcc_sb[:, cs])
                    nc.gpsimd.tensor_mul(rot, rot, tss_sb[:, cs])
                    nc.gpsimd.tensor_add(qr[g][:, cs], p1, rot)
                rot = rt.tile([128, TB], BF16, tag="rot", bufs=6)
                nc.scalar.dma_start(out=rot[0:32], in_=kvt[32:64, cs])
                nc.scalar.dma_start(out=rot[32:64], in_=kvt[0:32, cs])
                p1 = rt.tile([128, TB], BF16, tag="p1", bufs=6)
                nc.gpsimd.tensor_mul(p1[0:64], kvt[0:64, cs], tcc_sb[0:64, cs])
                nc.gpsimd.tensor_mul(rot[0:64], rot[0:64], tss_sb[0:64, cs])
                nc.gpsimd.tensor_add(kk[0:64, cs], p1[0:64], rot[0:64])
                nc.scalar.dma_start(out=kk[64:128, cs], in_=kk[0:64, cs])

                # V chunks via PE transpose
                for tt in range(4 * tb, 4 * tb + 4):
                    vp = psV.tile([128, 64], BF16, tag="vt", name=f"vp{tt}")
                    nc.tensor.transpose(
                        vp, kvt[64:128, tt * 128:(tt + 1) * 128],
                        ident[64:128, 64:128],
                    )
                    nc.vector.tensor_copy(v_sb[:, tt, 0:64], vp)

        # ---- Stage C: attention, software-pipelined; Wo matmuls as fillers
        with (
            tc.tile_pool(name="psS", bufs=2, space="PSUM") as psS,
            tc.tile_pool(name="psY", bufs=2, space="PSUM") as psY,
            tc.tile_pool(name="psO", bufs=2, space="PSUM") as psO,
            tc.tile_pool(name="esb", bufs=5) as esb,
            tc.tile_pool(name="rsb", bufs=2) as rsb,
            tc.tile_pool(name="osb", bufs=3) as osb,
        ):
            ready = []
            emitted = [0]

            def emit_d():
                tt, nb = ready.pop(0)
                tsl = slice(tt * 128, (tt + 1) * 128)
                nsl = slice(nb * TB, (nb + 1) * TB)
                po = psO.tile([128, TB], F32, tag="po", name=f"po{tt}_{nb}")
                nc.tensor.matmul(po, yt[0][:, tsl], wo_sb[:, 0, nsl],
                                 start=True, stop=False)
                nc.tensor.matmul(po, yt[1][:, tsl], wo_sb[:, 1, nsl],
                                 start=False, stop=True)
                ob = osb.tile([128, TB], BF16, tag="ob", name=f"ob{tt}_{nb}")
                # alternate the PSUM->SBUF cast between DVE and ACT so the
                # final drain isn't paced by a single engine
                if emitted[0] % 2 == 0:
                    nc.vector.tensor_copy(ob, po)
                else:
                    nc.scalar.copy(ob, po)
                emitted[0] += 1
                nc.sync.dma_start(out=out[tsl, nsl], in_=ob)

            for tb in range(NTB):
                qs = slice(tb * TB, (tb + 1) * TB)
                nts_here = 4 * tb + 4
                for g in range(PAIRS):
                    ya = psY.tile([65, TB], F32, tag="y", name=f"ya{g}_{tb}")
                    yb = psY.tile([65, TB], F32, tag="y", name=f"yb{g}_{tb}")
                    sps = {}
                    es = {}

                    def emit_s(ts):
                        # diagonal blocks (o>0) only need tq cols >= 128*o
                        o = max(0, ts - 4 * tb)
                        w = 128 * o
                        sub = slice(tb * TB + w, (tb + 1) * TB)
                        sp = psS.tile([128, 2 * TB], F32, tag="s",
                                      name=f"s{g}_{tb}_{ts}")
                        nc.tensor.matmul(
                            sp[:, w:TB], kk[0:64, ts * 128:(ts + 1) * 128],
                            qr[g][0:64, sub],
                            start=True, stop=True, tile_position=(0, 0),
                        )
                        nc.tensor.matmul(
                            sp[:, TB + w:2 * TB], kk[64:128, ts * 128:(ts + 1) * 128],
                            qr[g][64:128, sub],
                            start=True, stop=True, tile_position=(64, 0),
                        )
                        sps[ts] = sp

                    def emit_e(ts):
                        o = ts - 4 * tb
                        w = 128 * max(0, o)
                        sp = sps.pop(ts)
                        e = esb.tile([128, 2 * TB], BF16, tag="e",
                                     name=f"e{g}_{tb}_{ts}")
                        if w == 0:
                            nc.scalar.activation(e, sp, EXP, scale=SCALE)
                        else:
                            nc.scalar.activation(e[:, w:TB], sp[:, w:TB],
                                                 EXP, scale=SCALE)
                            nc.scalar.activation(e[:, TB + w:2 * TB],
                                                 sp[:, TB + w:2 * TB],
                                                 EXP, scale=SCALE)
                            # masked-out strips must read as zero in the AV
                            nc.gpsimd.memset(e[:, 0:w], 0.0)
                            nc.gpsimd.memset(e[:, TB:TB + w], 0.0)
                        if o >= 0:
                            # the partial 128-wide diagonal sub-block
                            nc.vector.tensor_mul(e[:, w:w + 128], e[:, w:w + 128],
                                                 bm_sb[:, 384:512])
                            nc.vector.tensor_mul(e[:, TB + w:TB + w + 128],
                                                 e[:, TB + w:TB + w + 128],
                                                 bm_sb[:, 384:512])
                        es[ts] = e

                    def emit_av(ts):
                        e = es.pop(ts)
                        st, last = ts == 0, ts == nts_here - 1
                        nc.tensor.matmul(ya, v_sb[:, ts], e[:, 0:TB],
                                         start=st, stop=last)
                        nc.tensor.matmul(yb, v_sb[:, ts], e[:, TB:2 * TB],
                                         start=st, stop=last)

                    # software pipeline: S/exp run two steps ahead of AV so
                    # ACT jitter never stalls the PE; Wo fillers issue before
                    # the exp-waiting AV
                    emit_s(0)
                    emit_e(0)
                    emit_s(1)
                    emit_e(1)
                    for ts in range(2, nts_here):
                        emit_s(ts)
                        emit_e(ts)
                        if ready:
                            emit_d()
                        emit_av(ts - 2)
                    if ready:
                        emit_d()
                    emit_av(nts_here - 2)
                    emit_av(nts_here - 1)

                    # normalize: PE-broadcast the raw denominator row, then
                    # one fast reciprocal of the whole broadcast (PSUM->SBUF)
                    # doubles as the copy; finally scale y into yt.
                    for h, yp in ((0, ya), (1, yb)):
                        l16 = rsb.tile([65, TB], BF16, tag="l16",
                                       name=f"l16{g}_{tb}_{h}")
                        nc.scalar.copy(l16[64:65], yp[64:65, :])
                        bc = psO.tile([64, TB], F32, tag="po",
                                      name=f"bc{g}_{tb}_{h}")
                        nc.tensor.matmul(bc, ones[64:65], l16[64:65],
                                         start=True, stop=True)
                        binv = rsb.tile([64, TB], F32, tag="binv",
                                        name=f"binv{g}_{tb}_{h}")
                        nc.vector.reciprocal_approx_fast(binv, bc)
                        if h == 0:
                            nc.vector.tensor_mul(yt[g][0:64, qs], yp[0:64, :], binv)
                        else:
                            yn = rsb.tile([64, TB], BF16, tag="yn",
                                          name=f"yn{g}_{tb}")
                            nc.vector.tensor_mul(yn, yp[0:64, :], binv)
                            nc.sync.dma_start(out=yt[g][64:128, qs], in_=yn)
                for tt in range(4 * tb, 4 * tb + 4):
                    for nb in range(NTB):
                        ready.append((tt, nb))
            while ready:
                emit_d()

    nc.compile()
    if not nc.is_finalized():
        nc.finalize()
    return nc


def _prep_inputs(x, rc, rs, Wq, Wk, Wv, Wo):
    bf16 = mybir.dt.np(BF16)
    xT = np.ascontiguousarray(x.reshape(T, D).T).astype(bf16)
    csT = np.ascontiguousarray(rc.T).astype(np.float32)   # [32, T]
    snT = np.ascontiguousarray(rs.T).astype(np.float32)
    tcc = np.ascontiguousarray(np.concatenate([csT, csT, csT, csT], 0)).astype(bf16)
    tss = np.ascontiguousarray(np.concatenate([-snT, snT, -snT, snT], 0)).astype(bf16)
    u = np.arange(896)[None, :]
    p = np.arange(128)[:, None]
    bm = (u >= p + 384).astype(bf16)

    in_maps = []
    for c in range(NCORES):
        wq_c = Wq[:, c * 256:(c + 1) * 256]               # [D, 256]
        wq_t = np.ascontiguousarray(
            wq_c.reshape(NKT, 128, PAIRS, 128).transpose(1, 2, 0, 3)
        ).astype(bf16)
        wkv_c = np.concatenate(
            [Wk[:, c * 64:(c + 1) * 64], Wv[:, c * 64:(c + 1) * 64]], 1
        )                                                  # [D, 128]
        wkv_t = np.ascontiguousarray(
            wkv_c.reshape(NKT, 128, 128).transpose(1, 0, 2)
        ).astype(bf16)
        wo_c = Wo[c * 256:(c + 1) * 256, :]                # [256, D]
        wo_t = np.ascontiguousarray(
            wo_c.reshape(2, 128, T).transpose(1, 0, 2)
        ).astype(bf16)
        in_maps.append(
            dict(xT=xT, wq=wq_t, wkv=wkv_t, wo=wo_t, tcc=tcc, tss=tss, bm=bm)
        )
    return in_maps


def kernel(x, rc, rs, Wq, Wk, Wv, Wo, _trace=False, _trace_kwargs=None):
    x = np.asarray(x, np.float32)
    if "nc" not in _CACHE:
        _CACHE["nc"] = build_nc()
    nc = _CACHE["nc"]
    in_maps = _prep_inputs(x, rc, rs, np.asarray(Wq), np.asarray(Wk),
                           np.asarray(Wv), np.asarray(Wo))
    kw = {}
    if _trace:
        kw = dict(trace=True, **(_trace_kwargs or {}))
    res = run_bass_kernel_spmd(nc, in_maps, list(range(NCORES)), **kw)
    parts = np.stack(
        [np.asarray(res.results[i]["out"]).astype(np.float32) for i in range(NCORES)]
    )
    full = parts.sum(0, dtype=np.float64).astype(np.float32)
    kernel.last_result = res
    return full.reshape(B, T, D)


# revision 33
# speedup vs baseline: 1.8333x; 1.1432x over previous
"""GQA attention (B=1, T=2048, D=2048, 32 q heads / 8 kv heads, DH=64, RoPE,
causal) on 8 Trainium2 NeuronCores, tensor-parallel over heads.

Per core: 1 kv head + its 4 q heads (2 pairs). Kernel computes, per core,
partial = (softmax(rope(Q) rope(K)^T / 8) V) @ Wo_shard ; host sums partials.

v2 layout/scheduling notes:
  - all matmul operands in bf16 (fp32 PSUM accumulate): same PE rate as
    fp32r but half the DMA/SBUF traffic and 2x DVE on elementwise ops
  - scores are built transposed: S^T[ts, tq] = K^T_chunk.T @ Q^T so the
    AV matmul needs no transposes; V' = [V | 1] gives the softmax
    denominator in row 64 of the AV accumulator
  - stage C is software-pipelined: S(ts+1) issues before AV(ts) so the
    exp latency hides behind PE work; Wo matmuls interleave as fillers
    and stream straight from PSUM to DRAM via DMA
  - normalize: reciprocal_approx_fast (DVE) + partition_broadcast
    (gpsimd) keeps the PE out of the broadcast business
"""

import numpy as np
from contextlib import ExitStack

import concourse.bass as bass
from concourse import bacc
import concourse.mybir as mybir
import concourse.tile as tile
from concourse.bass_utils import run_bass_kernel_spmd
from concourse.masks import make_identity

B, T, D = 1, 2048, 2048
NH, NKV, DH = 32, 8, 64
NCORES = 8
HPC = NH // NCORES      # 4 q heads per core
PAIRS = HPC // 2        # 2
TB = 512                # tq block (one psum bank of fp32)
NTB = T // TB           # 4
NKT = D // 128          # 16 contraction tiles
NTS = T // 128          # 16 ts blocks
SCALE = 1.0 / float(np.sqrt(DH))

F32 = mybir.dt.float32
BF16 = mybir.dt.bfloat16
EXP = mybir.ActivationFunctionType.Exp

_CACHE = {}


def build_nc():
    nc = bacc.Bacc(None, target_bir_lowering=False)

    xT = nc.declare_dram_parameter("xT", [D, T], BF16, isOutput=False)
    wq = nc.declare_dram_parameter("wq", [128, NKT, PAIRS, 128], BF16, isOutput=False)
    wkv = nc.declare_dram_parameter("wkv", [128, NKT, 128], BF16, isOutput=False)
    wo = nc.declare_dram_parameter("wo", [128, 2, T], BF16, isOutput=False)
    tcc = nc.declare_dram_parameter("tcc", [128, T], BF16, isOutput=False)
    tss = nc.declare_dram_parameter("tss", [128, T], BF16, isOutput=False)
    bm = nc.declare_dram_parameter("bm", [128, 896], BF16, isOutput=False)
    out = nc.declare_dram_parameter("out", [T, D], BF16, isOutput=True)

    with tile.TileContext(nc) as tc, ExitStack() as top:
        per = top.enter_context(tc.tile_pool(name="persist", bufs=1))

        wq_sb = per.tile([128, NKT, PAIRS, 128], BF16, tag="wq")
        wkv_sb = per.tile([128, NKT, 128], BF16, tag="wkv")
        wo_sb = per.tile([128, 2, T], BF16, tag="wo")
        tcc_sb = per.tile([128, T], BF16, tag="tcc")
        tss_sb = per.tile([128, T], BF16, tag="tss")
        bm_sb = per.tile([128, 896], BF16, tag="bm")

        qt = [per.tile([128, T], BF16, tag=f"qt{g}", name=f"qt{g}") for g in range(PAIRS)]
        kvt = per.tile([128, T], BF16, tag="kvt")
        qr = [per.tile([128, T], BF16, tag=f"qr{g}", name=f"qr{g}") for g in range(PAIRS)]
        kk = per.tile([128, T], BF16, tag="kk")
        v_sb = per.tile([128, NTS, 65], BF16, tag="v")
        yt = [per.tile([128, T], BF16, tag=f"yt{g}", name=f"yt{g}") for g in range(PAIRS)]

        # wq/wkv gate the first matmuls -- load them first, in kt-major halves
        # (contiguous per partition) so matmul kt=0 can start early; the rest
        # are staggered into the stage-A loop so x tiles aren't queued behind
        # them
        nc.sync.dma_start(out=wq_sb[:, 0:8], in_=wq[:, 0:8])
        nc.sync.dma_start(out=wkv_sb[:, 0:8], in_=wkv[:, 0:8])
        nc.sync.dma_start(out=wq_sb[:, 8:16], in_=wq[:, 8:16])
        nc.sync.dma_start(out=wkv_sb[:, 8:16], in_=wkv[:, 8:16])
        onesv = per.tile([128, NTS, 1], BF16, tag="onesv")
        nc.vector.memset(onesv, 1.0)
        nc.vector.tensor_copy(v_sb[:, :, 64:65], onesv)
        ones = per.tile([65, 64], BF16, tag="ones")
        nc.vector.memset(ones, 1.0)
        ident = per.tile([128, 128], BF16, tag="ident")
        make_identity(nc, ident)

        # ---- Stage A+B: QKV projections + RoPE + V transpose, per tq block
        with (
            tc.tile_pool(name="psA", bufs=6, space="PSUM") as psA,
            tc.tile_pool(name="psV", bufs=2, space="PSUM") as psV,
            tc.tile_pool(name="xs", bufs=12) as xs,
            tc.tile_pool(name="rtmp", bufs=2) as rt,
        ):
            for tb in range(NTB):
                accs = [psA.tile([128, TB], F32, tag="acc", name=f"acc{tb}_{i}")
                        for i in range(3)]
                for kt in range(NKT):
                    xt = xs.tile([128, TB], BF16, tag="x")
                    nc.sync.dma_start(
                        out=xt,
                        in_=xT[kt * 128:(kt + 1) * 128, tb * TB:(tb + 1) * TB],
                    )
                    st, sp = kt == 0, kt == NKT - 1
                    nc.tensor.matmul(accs[0], (wq_sb[:, kt, 0]), (xt), start=st, stop=sp)
                    nc.tensor.matmul(accs[1], (wq_sb[:, kt, 1]), (xt), start=st, stop=sp)
                    nc.tensor.matmul(accs[2], (wkv_sb[:, kt]), (xt), start=st, stop=sp)
                if tb == 0:
                    nc.sync.dma_start(out=tcc_sb, in_=tcc[:])
                    nc.sync.dma_start(out=tss_sb, in_=tss[:])
                elif tb == 1:
                    nc.sync.dma_start(out=bm_sb, in_=bm[:])
                    nc.sync.dma_start(out=wo_sb, in_=wo[:])
                cs = slice(tb * TB, (tb + 1) * TB)
                nc.scalar.copy(qt[0][:, cs], accs[0])
                nc.scalar.copy(qt[1][:, cs], accs[1])
                nc.vector.tensor_copy(kvt[:, cs], accs[2])

                # RoPE on this tq block: rotate-half via partition-swap DMA
                # (issued on the ACT DGE so stalled swaps never block the
                # sync-engine x-tile loads), qr = q*cc + rot(q)*ss with the
                # signs folded into ss.
                for g in range(PAIRS):
                    rot = rt.tile([128, TB], BF16, tag="rot", bufs=6)
                    for b in range(0, 128, 64):
                        nc.scalar.dma_start(out=rot[b:b + 32], in_=qt[g][b + 32:b + 64, cs])
                        nc.scalar.dma_start(out=rot[b + 32:b + 64], in_=qt[g][b:b + 32, cs])
                    p1 = rt.tile([128, TB], BF16, tag="p1", bufs=6)
                    nc.gpsimd.tensor_mul(p1, qt[g][:, cs], tcc_sb[:, cs])
                    nc.gpsimd.tensor_mul(rot, rot, tss_sb[:, cs])
                    nc.gpsimd.tensor_add(qr[g][:, cs], p1, rot)
                rot = rt.tile([128, TB], BF16, tag="rot", bufs=6)
                nc.scalar.dma_start(out=rot[0:32], in_=kvt[32:64, cs])
                nc.scalar.dma_start(out=rot[32:64], in_=kvt[0:32, cs])
                p1 = rt.tile([128, TB], BF16, tag="p1", bufs=6)
                nc.gpsimd.tensor_mul(p1[0:64], kvt[0:64, cs], tcc_sb[0:64, cs])
                nc.gpsimd.tensor_mul(rot[0:64], rot[0:64], tss_sb[0:64, cs])
                nc.gpsimd.tensor_add(kk[0:64, cs], p1[0:64], rot[0:64])
                nc.scalar.dma_start(out=kk[64:128, cs], in_=kk[0:64, cs])

                # V chunks via PE transpose
                for tt in range(4 * tb, 4 * tb + 4):
                    vp = psV.tile([128, 64], BF16, tag="vt", name=f"vp{tt}")
                    nc.tensor.transpose(
                        vp, kvt[64:128, tt * 128:(tt + 1) * 128],
                        ident[64:128, 64:128],
                    )
                    nc.vector.tensor_copy(v_sb[:, tt, 0:64], vp)

        # ---- Stage C: attention, software-pipelined; Wo matmuls as fillers
        with (
            tc.tile_pool(name="psS", bufs=2, space="PSUM") as psS,
            tc.tile_pool(name="psY", bufs=2, space="PSUM") as psY,
            tc.tile_pool(name="psO", bufs=2, space="PSUM") as psO,
            tc.tile_pool(name="esb", bufs=5) as esb,
            tc.tile_pool(name="rsb", bufs=2) as rsb,
            tc.tile_pool(name="osb", bufs=3) as osb,
        ):
            ready = []
            emitted = [0]

            def emit_d(pool=None, spread=False):
                tt, nb = ready.pop(0)
                tsl = slice(tt * 128, (tt + 1) * 128)
                nsl = slice(nb * TB, (nb + 1) * TB)
                po = (pool or psO).tile([128, TB], F32, tag="po",
                                        name=f"po{tt}_{nb}")
                nc.tensor.matmul(po, yt[0][:, tsl], wo_sb[:, 0, nsl],
                                 start=True, stop=False)
                nc.tensor.matmul(po, yt[1][:, tsl], wo_sb[:, 1, nsl],
                                 start=False, stop=True)
                ob = osb.tile([128, TB], BF16, tag="ob", name=f"ob{tt}_{nb}",
                              bufs=6)
                # in the drain (exp done, ACT idle) spread casts over both
                # engines; during attention keep ACT free for exp
                if spread and emitted[0] % 2 == 0:
                    nc.scalar.copy(ob, po)
                else:
                    nc.vector.tensor_copy(ob, po)
                emitted[0] += 1
                nc.sync.dma_start(out=out[tsl, nsl], in_=ob)

            for tb in range(NTB):
                qs = slice(tb * TB, (tb + 1) * TB)
                nts_here = 4 * tb + 4
                for g in range(PAIRS):
                    ya = psY.tile([65, TB], F32, tag="y", name=f"ya{g}_{tb}")
                    yb = psY.tile([65, TB], F32, tag="y", name=f"yb{g}_{tb}")
                    sps = {}
                    es = {}

                    def emit_s(ts):
                        # diagonal blocks (o>0) only need tq cols >= 128*o
                        o = max(0, ts - 4 * tb)
                        w = 128 * o
                        sub = slice(tb * TB + w, (tb + 1) * TB)
                        sp = psS.tile([128, 2 * TB], F32, tag="s",
                                      name=f"s{g}_{tb}_{ts}")
                        nc.tensor.matmul(
                            sp[:, w:TB], kk[0:64, ts * 128:(ts + 1) * 128],
                            qr[g][0:64, sub],
                            start=True, stop=True, tile_position=(0, 0),
                        )
                        nc.tensor.matmul(
                            sp[:, TB + w:2 * TB], kk[64:128, ts * 128:(ts + 1) * 128],
                            qr[g][64:128, sub],
                            start=True, stop=True, tile_position=(64, 0),
                        )
                        sps[ts] = sp

                    def emit_e(ts):
                        o = ts - 4 * tb
                        w = 128 * max(0, o)
                        sp = sps.pop(ts)
                        e = esb.tile([128, 2 * TB], BF16, tag="e",
                                     name=f"e{g}_{tb}_{ts}")
                        if w == 0:
                            nc.scalar.activation(e, sp, EXP, scale=SCALE)
                        else:
                            nc.scalar.activation(e[:, w:TB], sp[:, w:TB],
                                                 EXP, scale=SCALE)
                            nc.scalar.activation(e[:, TB + w:2 * TB],
                                                 sp[:, TB + w:2 * TB],
                                                 EXP, scale=SCALE)
                            # masked-out strips must read as zero in the AV
                            nc.gpsimd.memset(e[:, 0:w], 0.0)
                            nc.gpsimd.memset(e[:, TB:TB + w], 0.0)
                        if o >= 0:
                            # the partial 128-wide diagonal sub-block
                            nc.vector.tensor_mul(e[:, w:w + 128], e[:, w:w + 128],
                                                 bm_sb[:, 384:512])
                            nc.vector.tensor_mul(e[:, TB + w:TB + w + 128],
                                                 e[:, TB + w:TB + w + 128],
                                                 bm_sb[:, 384:512])
                        es[ts] = e

                    def emit_av(ts):
                        e = es.pop(ts)
                        st, last = ts == 0, ts == nts_here - 1
                        nc.tensor.matmul(ya, v_sb[:, ts], e[:, 0:TB],
                                         start=st, stop=last)
                        nc.tensor.matmul(yb, v_sb[:, ts], e[:, TB:2 * TB],
                                         start=st, stop=last)

                    # software pipeline: S/exp run two steps ahead of AV so
                    # ACT jitter never stalls the PE; Wo fillers issue before
                    # the exp-waiting AV
                    emit_s(0)
                    emit_e(0)
                    emit_s(1)
                    emit_e(1)
                    for ts in range(2, nts_here):
                        emit_s(ts)
                        emit_e(ts)
                        if ready:
                            emit_d()
                        emit_av(ts - 2)
                    if ready:
                        emit_d()
                    emit_av(nts_here - 2)
                    emit_av(nts_here - 1)

                    # normalize: PE-broadcast the raw denominator row, then
                    # one fast reciprocal of the whole broadcast (PSUM->SBUF)
                    # doubles as the copy; finally scale y into yt.
                    for h, yp in ((0, ya), (1, yb)):
                        l16 = rsb.tile([65, TB], BF16, tag="l16",
                                       name=f"l16{g}_{tb}_{h}")
                        nc.scalar.copy(l16[64:65], yp[64:65, :])
                        bc = psO.tile([64, TB], F32, tag="po",
                                      name=f"bc{g}_{tb}_{h}")
                        nc.tensor.matmul(bc, ones[64:65], l16[64:65],
                                         start=True, stop=True)
                        binv = rsb.tile([64, TB], F32, tag="binv",
                                        name=f"binv{g}_{tb}_{h}")
                        nc.vector.reciprocal_approx_fast(binv, bc)
                        if h == 0:
                            nc.vector.tensor_mul(yt[g][0:64, qs], yp[0:64, :], binv)
                        else:
                            yn = rsb.tile([64, TB], BF16, tag="yn",
                                          name=f"yn{g}_{tb}")
                            nc.vector.tensor_mul(yn, yp[0:64, :], binv)
                            nc.sync.dma_start(out=yt[g][64:128, qs], in_=yn)
                for tt in range(4 * tb, 4 * tb + 4):
                    for nb in range(NTB):
                        ready.append((tt, nb))
        # drain: attention PSUM pools are closed, use a deeper po ring
        with (
            tc.tile_pool(name="psO2", bufs=5, space="PSUM") as psO2,
            tc.tile_pool(name="osb2", bufs=6) as osb,
        ):
            while ready:
                emit_d(pool=psO2, spread=True)

    nc.compile()
    if not nc.is_finalized():
        nc.finalize()
    return nc


def _prep_inputs(x, rc, rs, Wq, Wk, Wv, Wo):
    bf16 = mybir.dt.np(BF16)
    xT = np.ascontiguousarray(x.reshape(T, D).T).astype(bf16)
    csT = np.ascontiguousarray(rc.T).astype(np.float32)   # [32, T]
    snT = np.ascontiguousarray(rs.T).astype(np.float32)
    tcc = np.ascontiguousarray(np.concatenate([csT, csT, csT, csT], 0)).astype(bf16)
    tss = np.ascontiguousarray(np.concatenate([-snT, snT, -snT, snT], 0)).astype(bf16)
    u = np.arange(896)[None, :]
    p = np.arange(128)[:, None]
    bm = (u >= p + 384).astype(bf16)

    in_maps = []
    for c in range(NCORES):
        wq_c = Wq[:, c * 256:(c + 1) * 256]               # [D, 256]
        wq_t = np.ascontiguousarray(
            wq_c.reshape(NKT, 128, PAIRS, 128).transpose(1, 0, 2, 3)
        ).astype(bf16)
        wkv_c = np.concatenate(
            [Wk[:, c * 64:(c + 1) * 64], Wv[:, c * 64:(c + 1) * 64]], 1
        )                                                  # [D, 128]
        wkv_t = np.ascontiguousarray(
            wkv_c.reshape(NKT, 128, 128).transpose(1, 0, 2)
        ).astype(bf16)
        wo_c = Wo[c * 256:(c + 1) * 256, :]                # [256, D]
        wo_t = np.ascontiguousarray(
            wo_c.reshape(2, 128, T).transpose(1, 0, 2)
        ).astype(bf16)
        in_maps.append(
            dict(xT=xT, wq=wq_t, wkv=wkv_t, wo=wo_t, tcc=tcc, tss=tss, bm=bm)
        )
    return in_maps


def kernel(x, rc, rs, Wq, Wk, Wv, Wo, _trace=False, _trace_kwargs=None):
    x = np.asarray(x, np.float32)
    if "nc" not in _CACHE:
        _CACHE["nc"] = build_nc()
    nc = _CACHE["nc"]
    in_maps = _prep_inputs(x, rc, rs, np.asarray(Wq), np.asarray(Wk),
                           np.asarray(Wv), np.asarray(Wo))
    kw = {}
    if _trace:
        kw = dict(trace=True, **(_trace_kwargs or {}))
    res = run_bass_kernel_spmd(nc, in_maps, list(range(NCORES)), **kw)
    parts = np.stack(
        [np.asarray(res.results[i]["out"]).astype(np.float32) for i in range(NCORES)]
    )
    full = parts.sum(0, dtype=np.float64).astype(np.float32)
    kernel.last_result = res
    return full.reshape(B, T, D)


# revision 36
# speedup vs baseline: 1.8386x; 1.0029x over previous
"""GQA attention (B=1, T=2048, D=2048, 32 q heads / 8 kv heads, DH=64, RoPE,
causal) on 8 Trainium2 NeuronCores, tensor-parallel over heads.

Per core: 1 kv head + its 4 q heads (2 pairs). Kernel computes, per core,
partial = (softmax(rope(Q) rope(K)^T / 8) V) @ Wo_shard ; host sums partials.

v2 layout/scheduling notes:
  - all matmul operands in bf16 (fp32 PSUM accumulate): same PE rate as
    fp32r but half the DMA/SBUF traffic and 2x DVE on elementwise ops
  - scores are built transposed: S^T[ts, tq] = K^T_chunk.T @ Q^T so the
    AV matmul needs no transposes; V' = [V | 1] gives the softmax
    denominator in row 64 of the AV accumulator
  - stage C is software-pipelined: S(ts+1) issues before AV(ts) so the
    exp latency hides behind PE work; Wo matmuls interleave as fillers
    and stream straight from PSUM to DRAM via DMA
  - normalize: reciprocal_approx_fast (DVE) + partition_broadcast
    (gpsimd) keeps the PE out of the broadcast business
"""

import numpy as np
from contextlib import ExitStack

import concourse.bass as bass
from concourse import bacc
import concourse.mybir as mybir
import concourse.tile as tile
from concourse.bass_utils import run_bass_kernel_spmd
from concourse.masks import make_identity

B, T, D = 1, 2048, 2048
NH, NKV, DH = 32, 8, 64
NCORES = 8
HPC = NH // NCORES      # 4 q heads per core
PAIRS = HPC // 2        # 2
TB = 512                # tq block (one psum bank of fp32)
NTB = T // TB           # 4
NKT = D // 128          # 16 contraction tiles
NTS = T // 128          # 16 ts blocks
SCALE = 1.0 / float(np.sqrt(DH))

F32 = mybir.dt.float32
BF16 = mybir.dt.bfloat16
EXP = mybir.ActivationFunctionType.Exp

_CACHE = {}


def build_nc():
    nc = bacc.Bacc(None, target_bir_lowering=False)

    xT = nc.declare_dram_parameter("xT", [D, T], BF16, isOutput=False)
    wq = nc.declare_dram_parameter("wq", [128, NKT, PAIRS, 128], BF16, isOutput=False)
    wkv = nc.declare_dram_parameter("wkv", [128, NKT, 128], BF16, isOutput=False)
    wo = nc.declare_dram_parameter("wo", [128, 2, T], BF16, isOutput=False)
    tcc = nc.declare_dram_parameter("tcc", [128, T], BF16, isOutput=False)
    tss = nc.declare_dram_parameter("tss", [128, T], BF16, isOutput=False)
    bm = nc.declare_dram_parameter("bm", [128, 896], BF16, isOutput=False)
    out = nc.declare_dram_parameter("out", [T, D], BF16, isOutput=True)

    with tile.TileContext(nc) as tc, ExitStack() as top:
        per = top.enter_context(tc.tile_pool(name="persist", bufs=1))

        wq_sb = per.tile([128, NKT, PAIRS, 128], BF16, tag="wq")
        wkv_sb = per.tile([128, NKT, 128], BF16, tag="wkv")
        wo_sb = per.tile([128, 2, T], BF16, tag="wo")
        tcc_sb = per.tile([128, T], BF16, tag="tcc")
        tss_sb = per.tile([128, T], BF16, tag="tss")
        bm_sb = per.tile([128, 896], BF16, tag="bm")

        qt = [per.tile([128, T], BF16, tag=f"qt{g}", name=f"qt{g}") for g in range(PAIRS)]
        kvt = per.tile([128, T], BF16, tag="kvt")
        qr = [per.tile([128, T], BF16, tag=f"qr{g}", name=f"qr{g}") for g in range(PAIRS)]
        kk = per.tile([128, T], BF16, tag="kk")
        v_sb = per.tile([128, NTS, 65], BF16, tag="v")
        yt = [per.tile([128, T], BF16, tag=f"yt{g}", name=f"yt{g}") for g in range(PAIRS)]

        # wq/wkv gate the first matmuls -- load them first, in kt-major halves
        # (contiguous per partition) so matmul kt=0 can start early; the rest
        # are staggered into the stage-A loop so x tiles aren't queued behind
        # them
        for c in range(8):
            nc.sync.dma_start(out=wq_sb[:, 2 * c:2 * c + 2], in_=wq[:, 2 * c:2 * c + 2])
        for c in range(4):
            nc.sync.dma_start(out=wkv_sb[:, 4 * c:4 * c + 4], in_=wkv[:, 4 * c:4 * c + 4])
        onesv = per.tile([128, NTS, 1], BF16, tag="onesv")
        nc.vector.memset(onesv, 1.0)
        nc.vector.tensor_copy(v_sb[:, :, 64:65], onesv)
        ones = per.tile([65, 64], BF16, tag="ones")
        nc.vector.memset(ones, 1.0)
        ident = per.tile([128, 128], BF16, tag="ident")
        make_identity(nc, ident)

        # ---- Stage A+B: QKV projections + RoPE + V transpose, per tq block
        with (
            tc.tile_pool(name="psA", bufs=6, space="PSUM") as psA,
            tc.tile_pool(name="psV", bufs=2, space="PSUM") as psV,
            tc.tile_pool(name="xs", bufs=12) as xs,
            tc.tile_pool(name="rtmp", bufs=2) as rt,
        ):
            for tb in range(NTB):
                accs = [psA.tile([128, TB], F32, tag="acc", name=f"acc{tb}_{i}")
                        for i in range(3)]
                for kt in range(NKT):
                    xt = xs.tile([128, TB], BF16, tag="x")
                    nc.sync.dma_start(
                        out=xt,
                        in_=xT[kt * 128:(kt + 1) * 128, tb * TB:(tb + 1) * TB],
                    )
                    st, sp = kt == 0, kt == NKT - 1
                    nc.tensor.matmul(accs[0], (wq_sb[:, kt, 0]), (xt), start=st, stop=sp)
                    nc.tensor.matmul(accs[1], (wq_sb[:, kt, 1]), (xt), start=st, stop=sp)
                    nc.tensor.matmul(accs[2], (wkv_sb[:, kt]), (xt), start=st, stop=sp)
                if tb == 0:
                    for c in range(2):
                        hs = slice(c * 1024, (c + 1) * 1024)
                        nc.sync.dma_start(out=tcc_sb[:, hs], in_=tcc[:, hs])
                        nc.sync.dma_start(out=tss_sb[:, hs], in_=tss[:, hs])
                    nc.sync.dma_start(out=bm_sb, in_=bm[:])
                elif tb == 1:
                    for c in range(8):
                        hs = slice(c * 256, (c + 1) * 256)
                        nc.sync.dma_start(out=wo_sb[:, :, hs], in_=wo[:, :, hs])
                cs = slice(tb * TB, (tb + 1) * TB)
                nc.scalar.copy(qt[0][:, cs], accs[0])
                nc.scalar.copy(qt[1][:, cs], accs[1])
                nc.vector.tensor_copy(kvt[:, cs], accs[2])

                # RoPE on this tq block: rotate-half via partition-swap DMA
                # (issued on the ACT DGE so stalled swaps never block the
                # sync-engine x-tile loads), qr = q*cc + rot(q)*ss with the
                # signs folded into ss.
                for g in range(PAIRS):
                    rot = rt.tile([128, TB], BF16, tag="rot", bufs=6)
                    for b in range(0, 128, 64):
                        nc.scalar.dma_start(out=rot[b:b + 32], in_=qt[g][b + 32:b + 64, cs])
                        nc.scalar.dma_start(out=rot[b + 32:b + 64], in_=qt[g][b:b + 32, cs])
                    p1 = rt.tile([128, TB], BF16, tag="p1", bufs=6)
                    nc.vector.tensor_mul(p1, qt[g][:, cs], tcc_sb[:, cs])
                    nc.vector.tensor_mul(rot, rot, tss_sb[:, cs])
                    nc.vector.tensor_add(qr[g][:, cs], p1, rot)
                rot = rt.tile([128, TB], BF16, tag="rot", bufs=6)
                nc.scalar.dma_start(out=rot[0:32], in_=kvt[32:64, cs])
                nc.scalar.dma_start(out=rot[32:64], in_=kvt[0:32, cs])
                p1 = rt.tile([128, TB], BF16, tag="p1", bufs=6)
                nc.vector.tensor_mul(p1[0:64], kvt[0:64, cs], tcc_sb[0:64, cs])
                nc.vector.tensor_mul(rot[0:64], rot[0:64], tss_sb[0:64, cs])
                nc.vector.tensor_add(kk[0:64, cs], p1[0:64], rot[0:64])
                nc.scalar.dma_start(out=kk[64:128, cs], in_=kk[0:64, cs])

                # V chunks via PE transpose
                for tt in range(4 * tb, 4 * tb + 4):
                    vp = psV.tile([128, 64], BF16, tag="vt", name=f"vp{tt}")
                    nc.tensor.transpose(
                        vp, kvt[64:128, tt * 128:(tt + 1) * 128],
                        ident[64:128, 64:128],
                    )
                    nc.vector.tensor_copy(v_sb[:, tt, 0:64], vp)

        # ---- Stage C: attention, software-pipelined; Wo matmuls as fillers
        with (
            tc.tile_pool(name="psS", bufs=2, space="PSUM") as psS,
            tc.tile_pool(name="psY", bufs=2, space="PSUM") as psY,
            tc.tile_pool(name="psO", bufs=2, space="PSUM") as psO,
            tc.tile_pool(name="esb", bufs=5) as esb,
            tc.tile_pool(name="rsb", bufs=2) as rsb,
            tc.tile_pool(name="osb", bufs=3) as osb,
        ):
            ready = []
            emitted = [0]

            def emit_d(pool=None, spread=False):
                tt, nb = ready.pop(0)
                tsl = slice(tt * 128, (tt + 1) * 128)
                nsl = slice(nb * TB, (nb + 1) * TB)
                po = (pool or psO).tile([128, TB], F32, tag="po",
                                        name=f"po{tt}_{nb}")
                nc.tensor.matmul(po, yt[0][:, tsl], wo_sb[:, 0, nsl],
                                 start=True, stop=False)
                nc.tensor.matmul(po, yt[1][:, tsl], wo_sb[:, 1, nsl],
                                 start=False, stop=True)
                ob = osb.tile([128, TB], BF16, tag="ob", name=f"ob{tt}_{nb}",
                              bufs=6)
                # in the drain (exp done, ACT idle) spread casts over both
                # engines; during attention keep ACT free for exp
                if spread and emitted[0] % 2 == 0:
                    nc.scalar.copy(ob, po)
                else:
                    nc.vector.tensor_copy(ob, po)
                emitted[0] += 1
                nc.sync.dma_start(out=out[tsl, nsl], in_=ob)

            for tb in range(NTB):
                qs = slice(tb * TB, (tb + 1) * TB)
                nts_here = 4 * tb + 4
                for g in range(PAIRS):
                    ya = psY.tile([65, TB], F32, tag="y", name=f"ya{g}_{tb}")
                    yb = psY.tile([65, TB], F32, tag="y", name=f"yb{g}_{tb}")
                    sps = {}
                    es = {}

                    def emit_s(ts):
                        # diagonal blocks (o>0) only need tq cols >= 128*o
                        o = max(0, ts - 4 * tb)
                        w = 128 * o
                        sub = slice(tb * TB + w, (tb + 1) * TB)
                        sp = psS.tile([128, 2 * TB], F32, tag="s",
                                      name=f"s{g}_{tb}_{ts}")
                        nc.tensor.matmul(
                            sp[:, w:TB], kk[0:64, ts * 128:(ts + 1) * 128],
                            qr[g][0:64, sub],
                            start=True, stop=True, tile_position=(0, 0),
                        )
                        nc.tensor.matmul(
                            sp[:, TB + w:2 * TB], kk[64:128, ts * 128:(ts + 1) * 128],
                            qr[g][64:128, sub],
                            start=True, stop=True, tile_position=(64, 0),
                        )
                        sps[ts] = sp

                    def emit_e(ts):
                        o = ts - 4 * tb
                        w = 128 * max(0, o)
                        sp = sps.pop(ts)
                        e = esb.tile([128, 2 * TB], BF16, tag="e",
                                     name=f"e{g}_{tb}_{ts}")
                        if w == 0:
                            nc.scalar.activation(e, sp, EXP, scale=SCALE)
                        else:
                            nc.scalar.activation(e[:, w:TB], sp[:, w:TB],
                                                 EXP, scale=SCALE)
                            nc.scalar.activation(e[:, TB + w:2 * TB],
                                                 sp[:, TB + w:2 * TB],
                                                 EXP, scale=SCALE)
                            # masked-out strips must read as zero in the AV
                            nc.gpsimd.memset(e[:, 0:w], 0.0)
                            nc.gpsimd.memset(e[:, TB:TB + w], 0.0)
                        if o >= 0:
                            # the partial 128-wide diagonal sub-block
                            nc.vector.tensor_mul(e[:, w:w + 128], e[:, w:w + 128],
                                                 bm_sb[:, 384:512])
                            nc.vector.tensor_mul(e[:, TB + w:TB + w + 128],
                                                 e[:, TB + w:TB + w + 128],
                                                 bm_sb[:, 384:512])
                        es[ts] = e

                    def emit_av(ts):
                        e = es.pop(ts)
                        st, last = ts == 0, ts == nts_here - 1
                        nc.tensor.matmul(ya, v_sb[:, ts], e[:, 0:TB],
                                         start=st, stop=last)
                        nc.tensor.matmul(yb, v_sb[:, ts], e[:, TB:2 * TB],
                                         start=st, stop=last)

                    # software pipeline: S/exp run two steps ahead of AV so
                    # ACT jitter never stalls the PE; Wo fillers issue before
                    # the exp-waiting AV
                    emit_s(0)
                    emit_e(0)
                    emit_s(1)
                    emit_e(1)
                    for ts in range(2, nts_here):
                        emit_s(ts)
                        emit_e(ts)
                        if ready:
                            emit_d()
                        emit_av(ts - 2)
                    if ready:
                        emit_d()
                    emit_av(nts_here - 2)
                    emit_av(nts_here - 1)

                    # normalize: PE-broadcast the raw denominator row, then
                    # one fast reciprocal of the whole broadcast (PSUM->SBUF)
                    # doubles as the copy; finally scale y into yt.
                    for h, yp in ((0, ya), (1, yb)):
                        l16 = rsb.tile([65, TB], BF16, tag="l16",
                                       name=f"l16{g}_{tb}_{h}")
                        nc.scalar.copy(l16[64:65], yp[64:65, :])
                        bc = psO.tile([64, TB], F32, tag="po",
                                      name=f"bc{g}_{tb}_{h}")
                        nc.tensor.matmul(bc, ones[64:65], l16[64:65],
                                         start=True, stop=True)
                        binv = rsb.tile([64, TB], F32, tag="binv",
                                        name=f"binv{g}_{tb}_{h}")
                        nc.vector.reciprocal_approx_fast(binv, bc)
                        if h == 0:
                            nc.vector.tensor_mul(yt[g][0:64, qs], yp[0:64, :], binv)
                        else:
                            yn = rsb.tile([64, TB], BF16, tag="yn",
                                          name=f"yn{g}_{tb}")
                            nc.vector.tensor_mul(yn, yp[0:64, :], binv)
                            nc.sync.dma_start(out=yt[g][64:128, qs], in_=yn)
                for tt in range(4 * tb, 4 * tb + 4):
                    for nb in range(NTB):
                        ready.append((tt, nb))
        # drain: attention PSUM pools are closed, use a deeper po ring
        with (
            tc.tile_pool(name="psO2", bufs=5, space="PSUM") as psO2,
            tc.tile_pool(name="osb2", bufs=6) as osb,
        ):
            while ready:
                emit_d(pool=psO2, spread=True)

    nc.compile()
    if not nc.is_finalized():
        nc.finalize()
    return nc


def _prep_inputs(x, rc, rs, Wq, Wk, Wv, Wo):
    bf16 = mybir.dt.np(BF16)
    xT = np.ascontiguousarray(x.reshape(T, D).T).astype(bf16)
    csT = np.ascontiguousarray(rc.T).astype(np.float32)   # [32, T]
    snT = np.ascontiguousarray(rs.T).astype(np.float32)
    tcc = np.ascontiguousarray(np.concatenate([csT, csT, csT, csT], 0)).astype(bf16)
    tss = np.ascontiguousarray(np.concatenate([-snT, snT, -snT, snT], 0)).astype(bf16)
    u = np.arange(896)[None, :]
    p = np.arange(128)[:, None]
    bm = (u >= p + 384).astype(bf16)

    in_maps = []
    for c in range(NCORES):
        wq_c = Wq[:, c * 256:(c + 1) * 256]               # [D, 256]
        wq_t = np.ascontiguousarray(
            wq_c.reshape(NKT, 128, PAIRS, 128).transpose(1, 0, 2, 3)
        ).astype(bf16)
        wkv_c = np.concatenate(
            [Wk[:, c * 64:(c + 1) * 64], Wv[:, c * 64:(c + 1) * 64]], 1
        )                                                  # [D, 128]
        wkv_t = np.ascontiguousarray(
            wkv_c.reshape(NKT, 128, 128).transpose(1, 0, 2)
        ).astype(bf16)
        wo_c = Wo[c * 256:(c + 1) * 256, :]                # [256, D]
        wo_t = np.ascontiguousarray(
            wo_c.reshape(2, 128, T).transpose(1, 0, 2)
        ).astype(bf16)
        in_maps.append(
            dict(xT=xT, wq=wq_t, wkv=wkv_t, wo=wo_t, tcc=tcc, tss=tss, bm=bm)
        )
    return in_maps


def kernel(x, rc, rs, Wq, Wk, Wv, Wo, _trace=False, _trace_kwargs=None):
    x = np.asarray(x, np.float32)
    if "nc" not in _CACHE:
        _CACHE["nc"] = build_nc()
    nc = _CACHE["nc"]
    in_maps = _prep_inputs(x, rc, rs, np.asarray(Wq), np.asarray(Wk),
                           np.asarray(Wv), np.asarray(Wo))
    kw = {}
    if _trace:
        kw = dict(trace=True, **(_trace_kwargs or {}))
    res = run_bass_kernel_spmd(nc, in_maps, list(range(NCORES)), **kw)
    parts = np.stack(
        [np.asarray(res.results[i]["out"]).astype(np.float32) for i in range(NCORES)]
    )
    full = parts.sum(0, dtype=np.float64).astype(np.float32)
    kernel.last_result = res
    return full.reshape(B, T, D)
